# revision 22
# baseline (speedup 1.0000x reference)
"""Trainium2 Bass kernel for nn_MDNSeqModel: LSTM + encoder recurrence with
MDN decoder, data-parallel over batch across 8 NeuronCores.

Batch 1024 is sharded 8 ways -> 128 batch columns per core. Gate matmuls run
batch-major (activations stationary, weights moving in 512-wide windows).
Per bank the accumulation is 3 fp8 DoubleRow matmuls: h-chunk pairs (0,1),
(2,3) and the fused (z,a) pair -- z is stored fp8 at 1/8 scale interleaved
with the (padded) a_t slice in one stream tile so [z_{t-1} | a_t] is a
single DR stationary AP. Gate PSUM lives in two 2-bank tiles so the four
tanh ACT instructions collapse to two ([f,i] and [g,o]); tanh/exp/relu/copy
share one ACT table set (sigmoid = 0.5+0.5*tanh(x/2), 0.5 pre-folded into
f/i/o weight rows). Elementwise state (tanh outs, c, h) is bf16 for DVE
throughput. State carried as cD = 2c and h2 = 2h. h2 is transposed to
feature-major on the PE; psum->sbuf copies split ACT/DVE. The decoder is
interleaved into the recurrence (one 512-column chunk every 4 steps,
emitted between the h-part and the transposes so its matmuls fill the PE
during the tanh/DVE phase); mu/sigma heads run as one stacked [16,42]
matmul. Final log-prob / mean reductions run on the host in float64.
"""
import os

import numpy as np
import ml_dtypes

STATE, ACT, Z, H = 21, 8, 128, 512
B, T = 1024, 128
NCORES = 8
BL = B // NCORES          # batch per core (free dim)
LOG_SQRT_2PI = 0.9189385332046727

bf16 = ml_dtypes.bfloat16
f8 = ml_dtypes.float8_e4m3

_CACHE = {}


def _split_multi_waits(nc, max_waits=1):
    """This walrus build rejects instructions carrying more than one sync-wait
    command; Tile's semaphore pass emits up to ~4 per instruction. Hoist the
    extras onto single-wait NOPs inserted just before, on the same engine
    (each engine executes its own stream in program order, so the semantics
    are identical)."""
    import concourse.mybir as mybir

    n_nops = 0
    for f in nc.m.functions:
        for bb in f.blocks:
            insts = bb.instructions
            out = []
            changed = False
            for ins in insts:
                si = ins.sync_info
                waits = list(si.on_wait) if si is not None else []
                if len(waits) > max_waits:
                    changed = True
                    extra = waits[:-max_waits]
                    for k, w in enumerate(extra):
                        nop = mybir.InstNoOp(
                            name=f"{ins.name}-wsplit{k}", engine=ins.engine)
                        nop.sync_info = mybir.SyncInfo(
                            on_update=[], on_wait=[w])
                        out.append(nop)
                        n_nops += 1
                    while len(si.on_wait) > max_waits:
                        si.on_wait.pop(0)
                out.append(ins)
            if changed:
                bb.instructions = out
    return n_nops


def _build_nc(t_steps):
    """Build the Bass module (same NEFF for all cores; SPMD over in_maps)."""
    import contextlib

    import concourse.bass as bass
    import concourse.mybir as mybir
    import concourse.tile as tile

    dt = mybir.dt
    AF = mybir.ActivationFunctionType
    OP = mybir.AluOpType
    DR = mybir.MatmulPerfMode.DoubleRow
    NT = t_steps * BL       # decoder free length

    nc = bass.Bass()
    P = nc.declare_dram_parameter

    # ---- inputs (per-core, host-prepped) ----
    wzah_d = P("wzah", [128, 2 * 4 * H], dt.float8e4, isOutput=False)
    wh_d = P("wh", [4, 128, 4 * H], dt.float8e4, isOutput=False)   # (32*W_hh)^T
    azx_d = P("azx", [128, (2 * t_steps + 2) * BL], dt.float8e4,
              isOutput=False)                                      # z/a stream
    w1_d = P("w1", [4, 128, 256], dt.float8e4, isOutput=False)     # (32*enc_w1)^T
    b1_d = P("b1", [128, 2], dt.float32, isOutput=False)
    w2_d = P("w2", [2, 128, 128], dt.float8e4, isOutput=False)     # (64*enc_w2)^T
    b2_d = P("b2", [128, 1], dt.float32, isOutput=False)           # 4096*b2
    wzz_d = P("wzz", [128, 2 * Z], dt.bfloat16, isOutput=False)    # [hi|lo]
    bzlo_d = P("bzlo", [Z, 1], dt.float32, isOutput=False)         # bz_lo/8
    bzhi_d = P("bzhi", [128, 1], dt.float32, isOutput=False)       # 1 + bz_hi
    dw1z_d = P("dw1z", [Z, 64], dt.float8e4, isOutput=False)       # 8*w
    dw1o_d = P("dw1o", [STATE + 1, 64], dt.bfloat16, isOutput=False)  # [w|b]
    dw2_d = P("dw2", [64, 64], dt.bfloat16, isOutput=False)
    db2_d = P("db2", [64, 1], dt.float32, isOutput=False)
    dw3_d = P("dw3", [64, 32], dt.bfloat16, isOutput=False)
    db3_d = P("db3", [32, 1], dt.float32, isOutput=False)
    dw4_d = P("dw4", [32, 16], dt.bfloat16, isOutput=False)
    db4_d = P("db4", [16, 1], dt.float32, isOutput=False)
    msw_d = P("msw", [16, 2 * STATE], dt.bfloat16, isOutput=False)  # [mu|sig]
    msb_d = P("msb", [2 * STATE, 1], dt.float32, isOutput=False)
    obs_d = P("obs_rep", [STATE + 1, 512], dt.bfloat16, isOutput=False)
    eps_d = P("eps", [t_steps, Z, BL], dt.bfloat16, isOutput=False)  # eps/8

    ms_o = P("ms_out", [2 * STATE, NT], dt.float32, isOutput=True)

    with tile.TileContext(nc) as tc, contextlib.ExitStack() as octx:
        wpool = octx.enter_context(tc.tile_pool(name="weights", bufs=1))
        spool = octx.enter_context(tc.tile_pool(name="state", bufs=1))
        gpool = octx.enter_context(tc.tile_pool(name="gwork", bufs=2))
        epool = octx.enter_context(tc.tile_pool(name="eps", bufs=4))
        dpool = octx.enter_context(tc.tile_pool(name="dwork", bufs=3))
        gpsum = octx.enter_context(
            tc.tile_pool(name="gpsum", bufs=1, space="PSUM"))
        epsum = octx.enter_context(
            tc.tile_pool(name="epsum", bufs=1, space="PSUM"))
        dpsum = octx.enter_context(
            tc.tile_pool(name="dpsum", bufs=3, space="PSUM"))

        # ---- load weights ----
        wzah = wpool.tile([128, 2 * 4 * H], dt.float8e4)
        wh = wpool.tile([128, 4 * 4 * H], dt.float8e4)
        w1 = wpool.tile([128, 4 * 256], dt.float8e4)
        w2 = wpool.tile([128, 2 * 128], dt.float8e4)
        wzz = wpool.tile([128, 2 * Z], dt.bfloat16)
        b1 = wpool.tile([128, 2], dt.float32)
        b2 = wpool.tile([128, 1], dt.float32)
        bzlo = wpool.tile([Z, 1], dt.float32)
        bzhi = wpool.tile([128, 1], dt.float32)
        nc.sync.dma_start(out=wzah[:], in_=wzah_d[:])
        for k in range(4):
            nc.sync.dma_start(
                out=wh[:, 2048 * k:2048 * (k + 1)], in_=wh_d[k, :, :])
            nc.sync.dma_start(
                out=w1[:, 256 * k:256 * (k + 1)], in_=w1_d[k, :, :])
        for k in range(2):
            nc.sync.dma_start(
                out=w2[:, 128 * k:128 * (k + 1)], in_=w2_d[k, :, :])
        nc.sync.dma_start(out=wzz[:], in_=wzz_d[:])
        nc.sync.dma_start(out=b1[:], in_=b1_d[:])
        nc.sync.dma_start(out=b2[:], in_=b2_d[:])
        nc.sync.dma_start(out=bzlo[:], in_=bzlo_d[:])
        nc.sync.dma_start(out=bzhi[:], in_=bzhi_d[:])

        # z/a stream: [z_{-1}=0 | a_0 | z_0 | a_1 | ... | z_{T-1} | pad],
        # fp8, z at 1/8; pair s = [z_{s-1} | a_s]
        azx = wpool.tile([128, (2 * t_steps + 2) * BL], dt.float8e4)
        nc.sync.dma_start(out=azx[:], in_=azx_d[:])
        azx2 = azx[:].rearrange("p (s two b) -> p s two b", two=2, b=BL)
        wzahv = wzah[:].rearrange("p (two w) -> p two w", two=2)

        # zse stream: slot s holds zse_{s-1}/8 in fp8; slot 0 = zeros
        zse_s = wpool.tile([128, (t_steps + 1) * BL], dt.float8e4)
        nc.vector.memset(zse_s[:, 0:BL], 0.0)

        # decoder weights
        dw1z = wpool.tile([Z, 64], dt.float8e4)
        dw1o = wpool.tile([STATE + 1, 64], dt.bfloat16)
        dw2 = wpool.tile([64, 64], dt.bfloat16)
        dw3 = wpool.tile([64, 32], dt.bfloat16)
        dw4 = wpool.tile([32, 16], dt.bfloat16)
        msw = wpool.tile([16, 2 * STATE], dt.bfloat16)
        db2 = wpool.tile([64, 1], dt.float32)
        db3 = wpool.tile([32, 1], dt.float32)
        db4 = wpool.tile([16, 1], dt.float32)
        msb = wpool.tile([2 * STATE, 1], dt.float32)
        obs = wpool.tile([STATE + 1, 512], dt.bfloat16)
        for tdst, tsrc in [(dw1z, dw1z_d), (dw1o, dw1o_d), (dw2, dw2_d),
                           (dw3, dw3_d), (dw4, dw4_d), (msw, msw_d),
                           (db2, db2_d), (db3, db3_d), (db4, db4_d),
                           (msb, msb_d), (obs, obs_d)]:
            nc.sync.dma_start(out=tdst[:], in_=tsrc[:])

        # ---- state ----
        h2b = spool.tile([128, H], dt.bfloat16)      # 2*h, batch-major
        h2f = spool.tile([128, H], dt.float8e4)      # 2*h, feature-major fp8
        cd = spool.tile([128, H], dt.bfloat16)       # 2*c, batch-major
        ident = spool.tile([128, 128], dt.bfloat16)
        nc.vector.memset(h2f[:], 0.0)
        nc.vector.memset(cd[:], 0.0)
        from concourse.masks import make_identity
        make_identity(nc, ident[:])

        # gates PSUM: two 2-bank tiles (f,i) and (g,o) so tanh runs as two
        # [128,1024] ACT instructions; WAR tracking still per 2-bank window
        pg = [gpsum.tile([128, 1024], dt.float32, tag=f"pg{n}",
                         name=f"pg{n}") for n in range(2)]

        def gbank(n):
            """psum region of gate bank n (0..3 = f,i,g,o)."""
            return pg[n // 2][:, 512 * (n % 2):512 * (n % 2 + 1)]

        dec_state = {}

        from bass_rust import add_dep_helper as _add_dep
        state = {}

        def _pin_dve(op):
            if "z_tt" in state:
                _add_dep(op.ins, state["z_tt"].ins, sync=False,
                         reason="decoder DVE stays behind the z chain")
            return op

        def _pin_mm(op):
            if "zz_mm" in state:
                _add_dep(op.ins, state["zz_mm"].ins, sync=False,
                         reason="decoder MM stays behind the encoder chain")
            return op

        def decoder_piece_mm(cidx, piece):
            """Decoder matmuls: emitted right after the h-part so they fill
            the PE while ACT/DVE run the gate tail. Piece 0 also carries the
            mu/sigma head matmul of the previous chunk (5 stages, 4 slots)."""
            if piece == 0:
                if cidx >= 1:
                    pm = dpsum.tile([2 * STATE, 512], dt.float32, tag="dscr")
                    _pin_mm(nc.tensor.matmul(
                        pm[:], msw[:], dec_state["d4"][:],
                        start=True, stop=True))
                    dec_state["pm"] = pm
                p1 = dpsum.tile([64, 512], dt.float32, tag="dscr")
                zblk = azx2[:, 4 * cidx + 1:4 * cidx + 5, 0, :]
                zseblk = zse_s[:, BL * (4 * cidx + 1):BL * (4 * cidx + 5)]
                _pin_mm(nc.tensor.matmul(p1[:], dw1z[:], zblk,
                                         start=True, stop=False))
                _pin_mm(nc.tensor.matmul(p1[:], dw1z[:], zseblk,
                                         start=False, stop=False))
                nc.tensor.matmul(p1[:], dw1o[:], obs[:],
                                 start=False, stop=True)
                dec_state["p"] = p1
            elif piece == 1:
                p2 = dpsum.tile([64, 512], dt.float32, tag="dscr")
                _pin_mm(nc.tensor.matmul(p2[:], dw2[:], dec_state["d1"][:],
                                         start=True, stop=True))
                dec_state["p"] = p2
            elif piece == 2:
                p3 = dpsum.tile([32, 512], dt.float32, tag="dscr")
                _pin_mm(nc.tensor.matmul(p3[:], dw3[:], dec_state["d2"][:],
                                         start=True, stop=True))
                dec_state["p"] = p3
            else:
                p4 = dpsum.tile([16, 512], dt.float32, tag="dscr")
                _pin_mm(nc.tensor.matmul(p4[:], dw4[:], dec_state["d3"][:],
                                         start=True, stop=True))
                dec_state["p"] = p4

        def decoder_piece_ew(cidx, piece):
            """Decoder elementwise: emitted after z_tt so the DVE in-order
            stream never blocks the gate tail behind a decoder matmul."""
            p = dec_state["p"]
            if piece == 0:
                if cidx >= 1:
                    cs = slice(512 * (cidx - 1), 512 * cidx)
                    ms_sb = dpool.tile([2 * STATE, 512], dt.float32,
                                       tag="ms")
                    _pin_dve(nc.vector.tensor_scalar_add(
                        ms_sb[:], dec_state["pm"][:], msb[:]))
                    nc.sync.dma_start(out=ms_o[:, cs], in_=ms_sb[:])
                d1 = dpool.tile([64, 512], dt.bfloat16, tag="d1")
                _pin_dve(nc.vector.tensor_scalar(
                    d1[:], p[:], 0.0, None, OP.max))
                dec_state["d1"] = d1
            elif piece == 1:
                d2 = dpool.tile([64, 512], dt.bfloat16, tag="d2")
                _pin_dve(nc.vector.tensor_scalar(
                    d2[:], p[:], db2[:], 0.0, OP.add, OP.max))
                dec_state["d2"] = d2
            elif piece == 2:
                d3 = dpool.tile([32, 512], dt.bfloat16, tag="d3")
                _pin_dve(nc.vector.tensor_scalar(
                    d3[:], p[:], db3[:], 0.0, OP.add, OP.max))
                dec_state["d3"] = d3
            else:
                d4 = dpool.tile([16, 512], dt.bfloat16, tag="d4")
                _pin_dve(nc.vector.tensor_scalar(
                    d4[:], p[:], db4[:], 0.0, OP.add, OP.max))
                dec_state["d4"] = d4

        def decoder_ms(cidx):
            pm = dpsum.tile([2 * STATE, 512], dt.float32, tag="dscr")
            nc.tensor.matmul(pm[:], msw[:], dec_state["d4"][:],
                             start=True, stop=True)
            ms_sb = dpool.tile([2 * STATE, 512], dt.float32, tag="ms")
            nc.vector.tensor_scalar_add(ms_sb[:], pm[:], msb[:])
            nc.sync.dma_start(out=ms_o[:, 512 * cidx:512 * (cidx + 1)],
                              in_=ms_sb[:])

        whv = wh[:].rearrange("p (k w) -> p k w", k=4)
        w1v = w1[:].rearrange("p (k w) -> p k w", k=4)

        def h_part(t):
            # h-part (fp8 DoubleRow: two 128-K-chunks per matmul); j==0
            # opens each bank's accumulation group; emitted right after the
            # copies so the PE streams it during the encoder z-path
            for j in range(2):
                stat = h2f[:, 256 * j:256 * (j + 1)].rearrange(
                    "p (two b) -> p two b", two=2)
                for n in range(4):
                    mm = nc.tensor.matmul(
                        gbank(n), stat,
                        whv[:, 2 * j:2 * (j + 1), 512 * n:512 * (n + 1)],
                        start=(j == 0), stop=False,
                        perf_mode=DR)
                    if n == 0 and j == 0 and "zz_mm" in state:
                        _add_dep(mm.ins, state["zz_mm"].ins, sync=False,
                                 reason="encoder chain before next h-part")

        def za_zse_part(t):
            # (zloc_{t-1}, a_t) DoubleRow group, then the zse closing
            # matmuls (stationary = zse stream slot; weights reuse the
            # z-half of wzah as a plain fp8 AP)
            stat = azx2[:, t, :, :]
            for n in range(4):
                nc.tensor.matmul(
                    gbank(n), stat,
                    wzahv[:, :, 512 * n:512 * (n + 1)],
                    start=False, stop=False,
                    perf_mode=DR)
            zstat = zse_s[:, BL * t:BL * (t + 1)]
            for n in range(4):
                nc.tensor.matmul(
                    gbank(n), zstat,
                    wzahv[:, 0, 512 * n:512 * (n + 1)],
                    start=False, stop=True)

        # ---- recurrence ----
        # gates(0): h2f, zloc_{-1}, zse_{-1} are all zeros
        h_part(0)
        za_zse_part(0)
        for t in range(t_steps):
            eps_t = epool.tile([Z, BL], dt.bfloat16, tag="eps")
            nc.sync.dma_start(out=eps_t[:], in_=eps_d[t, :, :])

            # tanh over (f,i) merged, then g, o; f,i,o pre-scaled by 0.5
            tg = gpool.tile([128, 4 * H], dt.float32, tag="tanh_g")
            nc.scalar.activation(tg[:, 0:1024], pg[0][:], AF.Tanh,
                                 scale=1.0 / 64.0)
            nc.scalar.activation(tg[:, 1024:1536], pg[1][:, 0:512],
                                 AF.Tanh, scale=1.0 / 64.0)
            nc.scalar.activation(tg[:, 1536:2048], pg[1][:, 512:1024],
                                 AF.Tanh, scale=1.0 / 64.0)

            t_f = tg[:, 0:512]
            t_i = tg[:, 512:1024]
            t_g = tg[:, 1024:1536]
            t_o = tg[:, 1536:2048]

            tmp1 = gpool.tile([128, H], dt.float32, tag="tmp1")
            tmp2 = gpool.tile([128, H], dt.float32, tag="tmp2")
            tcn = gpool.tile([128, H], dt.float32, tag="tanh_c")
            # tmp1 = (1+tanh(f/2)) * cD ; then per 256-half:
            # tmp2 = (1+tanh(i/2)) * g ; cD = 0.5*tmp1 + tmp2 (= 2*c_new)
            nc.vector.scalar_tensor_tensor(
                tmp1[:], t_f, 1.0, cd[:], OP.add, OP.mult)
            for hh in range(2):
                hs = slice(256 * hh, 256 * (hh + 1))
                nc.vector.scalar_tensor_tensor(
                    tmp2[:, hs], t_i[:, hs], 1.0, t_g[:, hs],
                    OP.add, OP.mult)
                nc.vector.scalar_tensor_tensor(
                    cd[:, hs], tmp1[:, hs], 0.5, tmp2[:, hs],
                    OP.mult, OP.add)
            # tc = tanh(0.5*cD); h2 = (1+tanh(o/2)) * tc, in halves so the
            # transposes / h-part start on the first half early
            for hh in range(2):
                hs = slice(256 * hh, 256 * (hh + 1))
                nc.scalar.activation(tcn[:, hs], cd[:, hs], AF.Tanh,
                                     scale=0.5)
                nc.vector.scalar_tensor_tensor(
                    h2b[:, hs], t_o[:, hs], 1.0, tcn[:, hs],
                    OP.add, OP.mult)

            # transpose h2 batch-major -> feature-major on the PE;
            # psum->sbuf copies split between ACT and DVE
            for k in range(4):
                ptr = dpsum.tile([128, 128], dt.bfloat16, tag="dscr")
                nc.tensor.transpose(
                    ptr[:], h2b[:, 128 * k:128 * (k + 1)], ident[:])
                dst = h2f[:, 128 * k:128 * (k + 1)]
                if k % 2 == 0:
                    nc.scalar.copy(dst, ptr[:])
                else:
                    nc.vector.tensor_copy(dst, ptr[:])

            # next step's h-part streams on the PE while the z-path runs
            if t + 1 < t_steps:
                h_part(t + 1)
            if t >= 4:
                decoder_piece_mm(t // 4 - 1, t % 4)

            # encoder (feature-major): e1/e2/zz share one PSUM bank;
            # e1 via fp8 DoubleRow on the same scaled operands
            pe = epsum.tile([128, 384], dt.float32, tag="enc")
            for m in range(2):
                out = pe[:, 128 * m:128 * (m + 1)]
                for j in range(2):
                    nc.tensor.matmul(
                        out,
                        w1v[:, 2 * j:2 * (j + 1), 128 * m:128 * (m + 1)],
                        h2f[:, 256 * j:256 * (j + 1)].rearrange(
                            "p (two b) -> p two b", two=2),
                        start=(j == 0), stop=(j == 1),
                        perf_mode=DR)
            # e1 stored as 64*e1 in fp8 (descale folded into enc_w2)
            e1 = gpool.tile([128, 256], dt.float8e4, tag="e1")
            nc.scalar.activation(e1[:, 0:128], pe[:, 0:128], AF.Relu,
                                 bias=b1[:, 0:1])
            nc.vector.tensor_scalar(
                e1[:, 128:256], pe[:, 128:256],
                b1[:, 1:2], 0.0, OP.add, OP.max)
            # e2' = max(psum + 4096*b2, 0) = 4096*e2 (descale in wzz)
            out = pe[:, 256:384]
            nc.tensor.matmul(
                out, w2[:].rearrange("p (k m) -> p k m", k=2),
                e1[:].rearrange("p (two b) -> p two b", two=2),
                start=True, stop=True,
                perf_mode=DR)
            e2 = gpool.tile([128, 128], dt.bfloat16, tag="e2")
            nc.vector.tensor_scalar(
                e2[:], out, b2[:], 0.0, OP.add, OP.max)
            # zz = WZ @ e2' (weights pre-descaled); hi first: the exp->zse
            # path is the longer consumer
            zz_mm = nc.tensor.matmul(pe[:, 0:Z], wzz[:, 0:Z], e2[:],
                                     start=True, stop=True)
            nc.tensor.matmul(pe[:, Z:2 * Z], wzz[:, Z:2 * Z], e2[:],
                             start=True, stop=True)
            state["zz_mm"] = zz_mm
            # z_scale = exp(zz_hi + (1+bz_hi))
            zsc = gpool.tile([Z, BL], dt.float32, tag="zsc")
            nc.scalar.activation(zsc[:], pe[:, 0:Z], AF.Exp,
                                 bias=bzhi[:])
            # zloc/8 into the z/a stream (off the exp path, runs on DVE in
            # parallel with the exp) -- z itself is never materialized:
            # gates and decoder consume zloc and zse separately
            nc.vector.tensor_scalar_add(
                azx2[:, t + 1, 0, :], pe[:, Z:2 * Z], bzlo[:])
            # zse/8 = zsc * eps/8 into the zse stream
            zse_op = nc.vector.tensor_tensor(
                zse_s[:, BL * (t + 1):BL * (t + 2)], zsc[:], eps_t[:],
                OP.mult)
            state["z_tt"] = zse_op

            # closing gate matmuls for the next step
            if t + 1 < t_steps:
                za_zse_part(t + 1)

            # decoder elementwise last: stays behind this step's chain
            if t >= 4:
                decoder_piece_ew(t // 4 - 1, t % 4)

        last = t_steps // 4 - 1
        decoder_ms(last - 1)
        for piece in range(4):
            decoder_piece_mm(last, piece)
            decoder_piece_ew(last, piece)
        decoder_ms(last)

    _split_multi_waits(nc)
    return nc


def _prep_host(inputs, t_steps):
    """Host-side weight/data prep -> per-core in_maps."""
    f32 = np.float32
    x, a = inputs["x"], inputs["a"]
    W_ih, W_hh = f32(inputs["W_ih"]), f32(inputs["W_hh"])
    b_g = f32(inputs["b_ih"]) + f32(inputs["b_hh"])

    # reorder gates (i,f,g,o) -> (f,i,g,o); scale f,i,o rows (and bias) by
    # 0.5 (g keeps scale 1: it gets a plain tanh)
    perm = np.concatenate([np.arange(H, 2 * H), np.arange(0, H),
                           np.arange(2 * H, 3 * H), np.arange(3 * H, 4 * H)])
    sc = np.ones(4 * H, f32)
    sc[:2 * H] = 0.5
    sc[3 * H:] = 0.5
    W_ih_r = W_ih[perm] * sc[:, None]
    W_hh_r = W_hh[perm] * sc[:, None]
    b_r = b_g[perm] * sc

    # the whole gate pre-activation is scaled by 64 (descaled in the tanh
    # via ACT scale); z is carried at 1/8 scale in fp8, so its weight rows
    # get an extra 8x
    wzah = np.zeros((128, 2, 4 * H), f32)
    wzah[:, 0, :] = 512.0 * W_ih_r[:, ACT:].T                      # z rows
    wzah[:ACT, 1, :] = 64.0 * W_ih_r[:, :ACT].T                    # a rows
    wzah[ACT, 1, :] = 64.0 * b_r                                   # bias row
    wzah = wzah.reshape(128, 2 * 4 * H).astype(f8)
    wh = np.ascontiguousarray(
        (32.0 * W_hh_r).T.reshape(4, 128, 4 * H)).astype(f8)
    w1 = np.ascontiguousarray(
        (32.0 * f32(inputs["enc_w1"])).T.reshape(4, 128, 256)).astype(f8)
    b1 = np.ascontiguousarray(64.0 * f32(inputs["enc_b1"]).reshape(2, 128).T)
    w2 = np.ascontiguousarray(
        (64.0 * f32(inputs["enc_w2"])).T.reshape(2, 128, 128)).astype(f8)
    b2 = (4096.0 * f32(inputs["enc_b2"])).reshape(128, 1)
    # zz consumes e2' = 4096*e2 -> weights /4096; lo half additionally /8
    enc_wz = f32(inputs["enc_wz"])
    wzz = np.concatenate([enc_wz[Z:, :] / 4096.0,
                          enc_wz[:Z, :] / (8.0 * 4096.0)], axis=0)
    wzz = np.ascontiguousarray(wzz.T).astype(bf16)                 # [128, 256]
    bzlo = (f32(inputs["enc_bz"])[:Z] / 8.0).reshape(Z, 1)
    bzhi = (1.0 + f32(inputs["enc_bz"])[Z:]).reshape(128, 1)

    dec_w1 = f32(inputs["dec_w1"])
    # z is carried at 1/8 scale -> 8x on its weight rows; obs part raw
    dw1z = np.ascontiguousarray(
        8.0 * dec_w1[:, STATE:].T).astype(f8)                      # [128, 64]
    dw1o = np.ascontiguousarray(
        np.concatenate([dec_w1[:, :STATE],
                        f32(inputs["dec_b1"])[:, None]],
                       axis=1).T).astype(bf16)                     # [22, 64]
    dw2 = np.ascontiguousarray(f32(inputs["dec_w2"]).T).astype(bf16)
    db2 = f32(inputs["dec_b2"]).reshape(64, 1)
    dw3 = np.ascontiguousarray(f32(inputs["dec_w3"]).T).astype(bf16)
    db3 = f32(inputs["dec_b3"]).reshape(32, 1)
    dw4 = np.ascontiguousarray(f32(inputs["dec_w4"]).T).astype(bf16)
    db4 = f32(inputs["dec_b4"]).reshape(16, 1)
    msw = np.ascontiguousarray(np.concatenate(
        [f32(inputs["mu_w"]), f32(inputs["sig_w"])], axis=0).T).astype(bf16)
    msb = np.concatenate(
        [f32(inputs["mu_b"]), 1.0 + f32(inputs["sig_b"])]).reshape(
        2 * STATE, 1)

    # eps via jax CPU (exact reference PRNG); carried at 1/8 scale
    import jax
    with jax.default_device(jax.devices("cpu")[0]):
        eps = np.asarray(jax.random.normal(
            jax.random.key(42), (T, B, Z), dtype=jax.numpy.float32))

    shared = dict(wzah=wzah, wh=wh, w1=w1, b1=b1, w2=w2, b2=b2, wzz=wzz,
                  bzlo=bzlo, bzhi=bzhi, dw1z=dw1z, dw1o=dw1o, dw2=dw2,
                  db2=db2, dw3=dw3, db3=db3, dw4=dw4, db4=db4, msw=msw,
                  msb=msb)

    in_maps = []
    for ci in range(NCORES):
        bs = slice(ci * BL, (ci + 1) * BL)
        # z/a stream: [z_{-1}=0 | a_0 | z_0 | a_1 | ...]; a_t slot holds
        # [a_t^T; 1; 0...] on 128 partitions, z slots start zeroed
        azx = np.zeros((128, 2 * t_steps + 2, BL), f32)
        a_c = f32(a[bs, :t_steps, :]).transpose(1, 2, 0)           # [T,ACT,BL]
        azx[:ACT, 1:2 * t_steps:2, :] = a_c.transpose(1, 0, 2)
        azx[ACT, 1:2 * t_steps:2, :] = 1.0
        azx = np.ascontiguousarray(
            azx.reshape(128, (2 * t_steps + 2) * BL)).astype(f8)
        eps_c = np.ascontiguousarray(
            (eps[:t_steps, bs, :] / 8.0).transpose(0, 2, 1)).astype(bf16)
        obs_c = f32(x[bs, 0, :]).T                                 # [21, BL]
        obs_rep = np.concatenate(
            [np.tile(obs_c, (1, 512 // BL)),
             np.ones((1, 512), f32)], axis=0).astype(bf16)
        m = dict(shared)
        m.update(azx=azx, eps=eps_c, obs_rep=obs_rep)
        in_maps.append(m)
    return in_maps


def _run(inputs, t_steps=T):
    from concourse.bass_utils import run_bass_kernel_spmd

    key = ("nc", t_steps)
    if key not in _CACHE:
        _CACHE[key] = _build_nc(t_steps)
    nc = _CACHE[key]
    in_maps = _prep_host(inputs, t_steps)
    res = run_bass_kernel_spmd(nc, in_maps, list(range(NCORES)),
                               trace=False)
    return res.results


def kernel(**inputs):
    t_steps = T
    results = _run(inputs, t_steps)

    y = np.float32(inputs["y"])
    su2 = 0.0
    ss = 0.0
    sabs = 0.0
    ssd = 0.0
    n_el = NCORES * STATE * t_steps * BL
    for ci in range(NCORES):
        bs = slice(ci * BL, (ci + 1) * BL)
        ms = results[ci]["ms_out"].astype(np.float64)     # [42, T*BL]
        mu = ms[:STATE]
        s = ms[STATE:]                                    # log(sigma)
        y_c = y[bs, :t_steps, :].transpose(2, 1, 0).reshape(
            STATE, t_steps * BL).astype(np.float64)
        sd = np.exp(s)
        u = (y_c - mu) / sd
        su2 += (u * u).sum()
        ss += s.sum()
        sabs += np.abs(mu - y_c).sum()
        ssd += sd.sum()
    n_tb = NCORES * t_steps * BL
    out1 = (0.5 * su2 + ss) / n_tb + STATE * LOG_SQRT_2PI
    out2 = sabs / n_el
    out3 = ssd / n_el
    return (np.float32(out1), np.float32(out2), np.float32(out3))


if __name__ == "__main__":
    import jax
    with jax.default_device(jax.devices("cpu")[0]):
        import reference as R
        inputs = {k: np.asarray(v) for k, v in R.setup_inputs().items()}
    out = kernel(**inputs)
    print("kernel:", [float(o) for o in out])


# revision 27
# speedup vs baseline: 1.2818x; 1.2818x over previous
"""Trainium2 Bass kernel for nn_MDNSeqModel: LSTM + encoder recurrence with
MDN decoder, data-parallel over batch across 8 NeuronCores.

Batch 1024 is sharded 8 ways -> 128 batch columns per core. Gate matmuls run
batch-major (activations stationary, weights moving in 512-wide windows).
Per bank the accumulation is 3 fp8 DoubleRow matmuls: h-chunk pairs (0,1),
(2,3) and the fused (z,a) pair -- z is stored fp8 at 1/8 scale interleaved
with the (padded) a_t slice in one stream tile so [z_{t-1} | a_t] is a
single DR stationary AP. Gate PSUM lives in two 2-bank tiles so the four
tanh ACT instructions collapse to two ([f,i] and [g,o]); tanh/exp/relu/copy
share one ACT table set (sigmoid = 0.5+0.5*tanh(x/2), 0.5 pre-folded into
f/i/o weight rows). Elementwise state (tanh outs, c, h) is bf16 for DVE
throughput. State carried as cD = 2c and h2 = 2h. h2 is transposed to
feature-major on the PE; psum->sbuf copies split ACT/DVE. The decoder is
interleaved into the recurrence (one 512-column chunk every 4 steps,
emitted between the h-part and the transposes so its matmuls fill the PE
during the tanh/DVE phase); mu/sigma heads run as one stacked [16,42]
matmul. Final log-prob / mean reductions run on the host in float64.
"""
import os

import numpy as np
import ml_dtypes

STATE, ACT, Z, H = 21, 8, 128, 512
B, T = 1024, 128
NCORES = 8
BL = B // NCORES          # batch per core (free dim)
LOG_SQRT_2PI = 0.9189385332046727

bf16 = ml_dtypes.bfloat16
f8 = ml_dtypes.float8_e4m3

_CACHE = {}


def _split_multi_waits(nc, max_waits=1):
    """This walrus build rejects instructions carrying more than one sync-wait
    command; Tile's semaphore pass emits up to ~4 per instruction. Hoist the
    extras onto single-wait NOPs inserted just before, on the same engine
    (each engine executes its own stream in program order, so the semantics
    are identical)."""
    import concourse.mybir as mybir

    n_nops = 0
    for f in nc.m.functions:
        for bb in f.blocks:
            insts = bb.instructions
            out = []
            changed = False
            for ins in insts:
                si = ins.sync_info
                waits = list(si.on_wait) if si is not None else []
                if len(waits) > max_waits:
                    changed = True
                    extra = waits[:-max_waits]
                    for k, w in enumerate(extra):
                        nop = mybir.InstNoOp(
                            name=f"{ins.name}-wsplit{k}", engine=ins.engine)
                        nop.sync_info = mybir.SyncInfo(
                            on_update=[], on_wait=[w])
                        out.append(nop)
                        n_nops += 1
                    while len(si.on_wait) > max_waits:
                        si.on_wait.pop(0)
                out.append(ins)
            if changed:
                bb.instructions = out
    return n_nops


def _build_nc(t_steps):
    """Build the Bass module (same NEFF for all cores; SPMD over in_maps)."""
    import contextlib

    import concourse.bass as bass
    import concourse.mybir as mybir
    import concourse.tile as tile

    dt = mybir.dt
    AF = mybir.ActivationFunctionType
    OP = mybir.AluOpType
    DR = mybir.MatmulPerfMode.DoubleRow
    NT = t_steps * BL       # decoder free length

    nc = bass.Bass()
    P = nc.declare_dram_parameter

    # ---- inputs (per-core, host-prepped) ----
    wzah_d = P("wzah", [128, 2 * 4 * H], dt.float8e4, isOutput=False)
    wh_d = P("wh", [4, 128, 4 * H], dt.float8e4, isOutput=False)   # (32*W_hh)^T
    azx_d = P("azx", [128, (2 * t_steps + 2) * BL], dt.float8e4,
              isOutput=False)                                      # z/a stream
    w1_d = P("w1", [4, 128, 256], dt.float8e4, isOutput=False)     # (32*enc_w1)^T
    b1_d = P("b1", [128, 2], dt.float32, isOutput=False)
    w2_d = P("w2", [2, 128, 128], dt.float8e4, isOutput=False)     # (64*enc_w2)^T
    b2_d = P("b2", [128, 1], dt.float32, isOutput=False)           # 4096*b2
    wzz_d = P("wzz", [128, 2 * Z], dt.bfloat16, isOutput=False)    # [hi|lo]
    bzlo_d = P("bzlo", [Z, 1], dt.float32, isOutput=False)         # bz_lo/8
    bzhi_d = P("bzhi", [128, 1], dt.float32, isOutput=False)       # 1 + bz_hi
    dw1z_d = P("dw1z", [Z, 64], dt.float8e4, isOutput=False)       # 8*w
    dw1o_d = P("dw1o", [STATE + 1, 64], dt.bfloat16, isOutput=False)  # [w|b]
    dw2_d = P("dw2", [64, 64], dt.bfloat16, isOutput=False)
    db2_d = P("db2", [64, 1], dt.float32, isOutput=False)
    dw3_d = P("dw3", [64, 32], dt.bfloat16, isOutput=False)
    db3_d = P("db3", [32, 1], dt.float32, isOutput=False)
    dw4_d = P("dw4", [32, 16], dt.bfloat16, isOutput=False)
    db4_d = P("db4", [16, 1], dt.float32, isOutput=False)
    msw_d = P("msw", [16, 2 * STATE], dt.bfloat16, isOutput=False)  # [mu|sig]
    msb_d = P("msb", [2 * STATE, 1], dt.float32, isOutput=False)
    obs_d = P("obs_rep", [STATE + 1, 512], dt.bfloat16, isOutput=False)
    eps_d = P("eps", [t_steps, Z, BL], dt.bfloat16, isOutput=False)  # eps/8

    ms_o = P("ms_out", [2 * STATE, NT], dt.float32, isOutput=True)

    with tile.TileContext(nc) as tc, contextlib.ExitStack() as octx:
        wpool = octx.enter_context(tc.tile_pool(name="weights", bufs=1))
        spool = octx.enter_context(tc.tile_pool(name="state", bufs=1))
        gpool = octx.enter_context(tc.tile_pool(name="gwork", bufs=2))
        epool = octx.enter_context(tc.tile_pool(name="eps", bufs=4))
        dpool = octx.enter_context(tc.tile_pool(name="dwork", bufs=3))
        gpsum = octx.enter_context(
            tc.tile_pool(name="gpsum", bufs=1, space="PSUM"))
        epsum = octx.enter_context(
            tc.tile_pool(name="epsum", bufs=1, space="PSUM"))
        dpsum = octx.enter_context(
            tc.tile_pool(name="dpsum", bufs=3, space="PSUM"))

        # ---- load weights ----
        wzah = wpool.tile([128, 2 * 4 * H], dt.float8e4)
        wh = wpool.tile([128, 4 * 4 * H], dt.float8e4)
        w1 = wpool.tile([128, 4 * 256], dt.float8e4)
        w2 = wpool.tile([128, 2 * 128], dt.float8e4)
        wzz = wpool.tile([128, 2 * Z], dt.bfloat16)
        b1 = wpool.tile([128, 2], dt.float32)
        b2 = wpool.tile([128, 1], dt.float32)
        bzlo = wpool.tile([Z, 1], dt.float32)
        bzhi = wpool.tile([128, 1], dt.float32)
        nc.sync.dma_start(out=wzah[:], in_=wzah_d[:])
        for k in range(4):
            nc.sync.dma_start(
                out=wh[:, 2048 * k:2048 * (k + 1)], in_=wh_d[k, :, :])
            nc.sync.dma_start(
                out=w1[:, 256 * k:256 * (k + 1)], in_=w1_d[k, :, :])
        for k in range(2):
            nc.sync.dma_start(
                out=w2[:, 128 * k:128 * (k + 1)], in_=w2_d[k, :, :])
        nc.sync.dma_start(out=wzz[:], in_=wzz_d[:])
        nc.sync.dma_start(out=b1[:], in_=b1_d[:])
        nc.sync.dma_start(out=b2[:], in_=b2_d[:])
        nc.sync.dma_start(out=bzlo[:], in_=bzlo_d[:])
        nc.sync.dma_start(out=bzhi[:], in_=bzhi_d[:])

        # z/a stream: [z_{-1}=0 | a_0 | z_0 | a_1 | ... | z_{T-1} | pad],
        # fp8, z at 1/8; pair s = [z_{s-1} | a_s]
        azx = wpool.tile([128, (2 * t_steps + 2) * BL], dt.float8e4)
        nc.sync.dma_start(out=azx[:], in_=azx_d[:])
        azx2 = azx[:].rearrange("p (s two b) -> p s two b", two=2, b=BL)
        wzahv = wzah[:].rearrange("p (two w) -> p two w", two=2)



        # decoder weights
        dw1z = wpool.tile([Z, 64], dt.float8e4)
        dw1o = wpool.tile([STATE + 1, 64], dt.bfloat16)
        dw2 = wpool.tile([64, 64], dt.bfloat16)
        dw3 = wpool.tile([64, 32], dt.bfloat16)
        dw4 = wpool.tile([32, 16], dt.bfloat16)
        msw = wpool.tile([16, 2 * STATE], dt.bfloat16)
        db2 = wpool.tile([64, 1], dt.float32)
        db3 = wpool.tile([32, 1], dt.float32)
        db4 = wpool.tile([16, 1], dt.float32)
        msb = wpool.tile([2 * STATE, 1], dt.float32)
        obs = wpool.tile([STATE + 1, 512], dt.bfloat16)
        for tdst, tsrc in [(dw1z, dw1z_d), (dw1o, dw1o_d), (dw2, dw2_d),
                           (dw3, dw3_d), (dw4, dw4_d), (msw, msw_d),
                           (db2, db2_d), (db3, db3_d), (db4, db4_d),
                           (msb, msb_d), (obs, obs_d)]:
            nc.sync.dma_start(out=tdst[:], in_=tsrc[:])

        # ---- state ----
        h2b = spool.tile([128, H], dt.bfloat16)      # 2*h, batch-major
        h2f = spool.tile([128, H], dt.float8e4)      # 2*h, feature-major fp8
        cd = spool.tile([128, H], dt.bfloat16)       # 2*c, batch-major
        ident = spool.tile([128, 128], dt.bfloat16)
        nc.vector.memset(h2f[:], 0.0)
        nc.vector.memset(cd[:], 0.0)
        from concourse.masks import make_identity
        make_identity(nc, ident[:])

        # gates PSUM: two 2-bank tiles (f,i) and (g,o) so tanh runs as two
        # [128,1024] ACT instructions; WAR tracking still per 2-bank window
        pg = [gpsum.tile([128, 1024], dt.float32, tag=f"pg{n}",
                         name=f"pg{n}") for n in range(2)]

        def gbank(n):
            """psum region of gate bank n (0..3 = f,i,g,o)."""
            return pg[n // 2][:, 512 * (n % 2):512 * (n % 2 + 1)]

        dec_state = {}

        from bass_rust import add_dep_helper as _add_dep
        state = {}

        def _pin_dve(op):
            if "z_tt" in state:
                _add_dep(op.ins, state["z_tt"].ins, sync=False,
                         reason="decoder DVE stays behind the z chain")
            return op

        def _pin_mm(op):
            if "zz_mm" in state:
                _add_dep(op.ins, state["zz_mm"].ins, sync=False,
                         reason="decoder MM stays behind the encoder chain")
            return op

        def decoder_piece_mm(cidx, piece):
            """Decoder matmuls: emitted right after the h-part so they fill
            the PE while ACT/DVE run the gate tail. Piece 0 also carries the
            mu/sigma head matmul of the previous chunk (5 stages, 4 slots)."""
            if piece == 0:
                if cidx >= 1:
                    pm = dpsum.tile([2 * STATE, 512], dt.float32, tag="dscr")
                    _pin_mm(nc.tensor.matmul(
                        pm[:], msw[:], dec_state["d4"][:],
                        start=True, stop=True))
                    dec_state["pm"] = pm
                p1 = dpsum.tile([64, 512], dt.float32, tag="dscr")
                zblk = azx2[:, 4 * cidx + 1:4 * cidx + 5, 0, :]
                _pin_mm(nc.tensor.matmul(p1[:], dw1z[:], zblk,
                                         start=True, stop=False))
                nc.tensor.matmul(p1[:], dw1o[:], obs[:],
                                 start=False, stop=True)
                dec_state["p"] = p1
            elif piece == 1:
                p2 = dpsum.tile([64, 512], dt.float32, tag="dscr")
                _pin_mm(nc.tensor.matmul(p2[:], dw2[:], dec_state["d1"][:],
                                         start=True, stop=True))
                dec_state["p"] = p2
            elif piece == 2:
                p3 = dpsum.tile([32, 512], dt.float32, tag="dscr")
                _pin_mm(nc.tensor.matmul(p3[:], dw3[:], dec_state["d2"][:],
                                         start=True, stop=True))
                dec_state["p"] = p3
            else:
                p4 = dpsum.tile([16, 512], dt.float32, tag="dscr")
                _pin_mm(nc.tensor.matmul(p4[:], dw4[:], dec_state["d3"][:],
                                         start=True, stop=True))
                dec_state["p"] = p4

        def decoder_piece_ew(cidx, piece):
            """Decoder elementwise: emitted after z_tt so the DVE in-order
            stream never blocks the gate tail behind a decoder matmul."""
            p = dec_state["p"]
            if piece == 0:
                if cidx >= 1:
                    cs = slice(512 * (cidx - 1), 512 * cidx)
                    ms_sb = dpool.tile([2 * STATE, 512], dt.float32,
                                       tag="ms")
                    _pin_dve(nc.vector.tensor_scalar_add(
                        ms_sb[:], dec_state["pm"][:], msb[:]))
                    nc.sync.dma_start(out=ms_o[:, cs], in_=ms_sb[:])
                d1 = dpool.tile([64, 512], dt.bfloat16, tag="d1")
                _pin_dve(nc.vector.tensor_scalar(
                    d1[:], p[:], 0.0, None, OP.max))
                dec_state["d1"] = d1
            elif piece == 1:
                d2 = dpool.tile([64, 512], dt.bfloat16, tag="d2")
                _pin_dve(nc.vector.tensor_scalar(
                    d2[:], p[:], db2[:], 0.0, OP.add, OP.max))
                dec_state["d2"] = d2
            elif piece == 2:
                d3 = dpool.tile([32, 512], dt.bfloat16, tag="d3")
                _pin_dve(nc.vector.tensor_scalar(
                    d3[:], p[:], db3[:], 0.0, OP.add, OP.max))
                dec_state["d3"] = d3
            else:
                d4 = dpool.tile([16, 512], dt.bfloat16, tag="d4")
                _pin_dve(nc.vector.tensor_scalar(
                    d4[:], p[:], db4[:], 0.0, OP.add, OP.max))
                dec_state["d4"] = d4

        def decoder_ms(cidx):
            pm = dpsum.tile([2 * STATE, 512], dt.float32, tag="dscr")
            nc.tensor.matmul(pm[:], msw[:], dec_state["d4"][:],
                             start=True, stop=True)
            ms_sb = dpool.tile([2 * STATE, 512], dt.float32, tag="ms")
            nc.vector.tensor_scalar_add(ms_sb[:], pm[:], msb[:])
            nc.sync.dma_start(out=ms_o[:, 512 * cidx:512 * (cidx + 1)],
                              in_=ms_sb[:])

        whv = wh[:].rearrange("p (k w) -> p k w", k=4)
        w1v = w1[:].rearrange("p (k w) -> p k w", k=4)

        def h_mm(t, j, n, start=False, stop=False):
            stat = h2f[:, 256 * j:256 * (j + 1)].rearrange(
                "p (two b) -> p two b", two=2)
            return nc.tensor.matmul(
                gbank(n), stat,
                whv[:, 2 * j:2 * (j + 1), 512 * n:512 * (n + 1)],
                start=start, stop=stop,
                perf_mode=DR)

        def az_mm(t, n, stop=False):
            # fused (z_{t-1}, a_t) DoubleRow accumulation
            return nc.tensor.matmul(
                gbank(n), azx2[:, t, :, :],
                wzahv[:, :, 512 * n:512 * (n + 1)],
                start=False, stop=stop,
                perf_mode=DR)

        # ---- recurrence ----
        # Per-bank accumulation groups: h j0 opens (start); banks f,i close
        # on their az matmul, banks g,o close on the h j1 matmul which is
        # DEFERRED into the next body so it streams during tanh(f,i) and
        # keeps the PE from going cold (p-state) in the gate tail.
        # gates(0): h2f and z_{-1} are zero
        for n in range(4):
            h_mm(0, 0, n, start=True)
        h_mm(0, 1, 0)
        h_mm(0, 1, 1)
        for n in range(4):
            az_mm(0, n, stop=(n < 2))
        for t in range(t_steps):
            eps_t = epool.tile([Z, BL], dt.bfloat16, tag="eps")
            nc.sync.dma_start(out=eps_t[:], in_=eps_d[t, :, :])

            # tanh over (f,i) merged; banks g,o close during it (deferred
            # h j1 matmuls keep the PE busy through the ACT phase)
            tg = gpool.tile([128, 4 * H], dt.float32, tag="tanh_g")
            nc.scalar.activation(tg[:, 0:1024], pg[0][:], AF.Tanh,
                                 scale=1.0 / 64.0)
            h_mm(t, 1, 2, stop=True)
            h_mm(t, 1, 3, stop=True)
            nc.scalar.activation(tg[:, 1024:1536], pg[1][:, 0:512],
                                 AF.Tanh, scale=1.0 / 64.0)
            nc.scalar.activation(tg[:, 1536:2048], pg[1][:, 512:1024],
                                 AF.Tanh, scale=1.0 / 64.0)

            t_f = tg[:, 0:512]
            t_i = tg[:, 512:1024]
            t_g = tg[:, 1024:1536]
            t_o = tg[:, 1536:2048]

            tmp1 = gpool.tile([128, H], dt.float32, tag="tmp1")
            tmp2 = gpool.tile([128, H], dt.float32, tag="tmp2")
            tcn = gpool.tile([128, H], dt.float32, tag="tanh_c")
            # tmp1 = (1+tanh(f/2)) * cD ; then per 256-half:
            # tmp2 = (1+tanh(i/2)) * g ; cD = 0.5*tmp1 + tmp2 (= 2*c_new)
            nc.vector.scalar_tensor_tensor(
                tmp1[:], t_f, 1.0, cd[:], OP.add, OP.mult)
            for hh in range(2):
                hs = slice(256 * hh, 256 * (hh + 1))
                nc.vector.scalar_tensor_tensor(
                    tmp2[:, hs], t_i[:, hs], 1.0, t_g[:, hs],
                    OP.add, OP.mult)
                nc.vector.scalar_tensor_tensor(
                    cd[:, hs], tmp1[:, hs], 0.5, tmp2[:, hs],
                    OP.mult, OP.add)
            # tc = tanh(0.5*cD); h2 = (1+tanh(o/2)) * tc, in halves so the
            # transposes / h-part start on the first half early
            for hh in range(2):
                hs = slice(256 * hh, 256 * (hh + 1))
                nc.scalar.activation(tcn[:, hs], cd[:, hs], AF.Tanh,
                                     scale=0.5)
                nc.vector.scalar_tensor_tensor(
                    h2b[:, hs], t_o[:, hs], 1.0, tcn[:, hs],
                    OP.add, OP.mult)

            # transpose h2 batch-major -> feature-major on the PE;
            # psum->sbuf copies split between ACT and DVE
            for k in range(4):
                ptr = dpsum.tile([128, 128], dt.bfloat16, tag="dscr")
                nc.tensor.transpose(
                    ptr[:], h2b[:, 128 * k:128 * (k + 1)], ident[:])
                dst = h2f[:, 128 * k:128 * (k + 1)]
                if k % 2 == 0:
                    nc.scalar.copy(dst, ptr[:])
                else:
                    nc.vector.tensor_copy(dst, ptr[:])

            # encoder (feature-major): e1/e2/zz share one PSUM bank; e1 via
            # fp8 DoubleRow; h(t+1) j0 matmuls interleave into the z-path's
            # PE wait slots so the PE never idles long enough to re-throttle
            pe = epsum.tile([128, 384], dt.float32, tag="enc")
            for m in range(2):
                out = pe[:, 128 * m:128 * (m + 1)]
                for j in range(2):
                    nc.tensor.matmul(
                        out,
                        w1v[:, 2 * j:2 * (j + 1), 128 * m:128 * (m + 1)],
                        h2f[:, 256 * j:256 * (j + 1)].rearrange(
                            "p (two b) -> p two b", two=2),
                        start=(j == 0), stop=(j == 1),
                        perf_mode=DR)
            # e1 stored as 64*e1 in fp8 (descale folded into enc_w2)
            e1 = gpool.tile([128, 256], dt.float8e4, tag="e1")
            nc.scalar.activation(e1[:, 0:128], pe[:, 0:128], AF.Relu,
                                 bias=b1[:, 0:1])
            nc.vector.tensor_scalar(
                e1[:, 128:256], pe[:, 128:256],
                b1[:, 1:2], 0.0, OP.add, OP.max)
            if t + 1 < t_steps:
                for n in range(4):
                    h_mm(t + 1, 0, n, start=True)
            # e2' = max(psum + 4096*b2, 0) = 4096*e2 (descale in wzz)
            out = pe[:, 256:384]
            nc.tensor.matmul(
                out, w2[:].rearrange("p (k m) -> p k m", k=2),
                e1[:].rearrange("p (two b) -> p two b", two=2),
                start=True, stop=True,
                perf_mode=DR)
            e2 = gpool.tile([128, 128], dt.bfloat16, tag="e2")
            nc.vector.tensor_scalar(
                e2[:], out, b2[:], 0.0, OP.add, OP.max)
            if t + 1 < t_steps:
                h_mm(t + 1, 1, 0)
                h_mm(t + 1, 1, 1)
            # zz = WZ @ e2' (weights pre-descaled); hi first: the exp->zse
            # path is the longer consumer
            zz_mm = nc.tensor.matmul(pe[:, 0:Z], wzz[:, 0:Z], e2[:],
                                     start=True, stop=True)
            nc.tensor.matmul(pe[:, Z:2 * Z], wzz[:, Z:2 * Z], e2[:],
                             start=True, stop=True)
            state["zz_mm"] = zz_mm
            if t >= 4:
                decoder_piece_mm(t // 4 - 1, t % 4)
            # z_scale = exp(zz_hi + (1+bz_hi))
            zsc = gpool.tile([Z, BL], dt.float32, tag="zsc")
            nc.scalar.activation(zsc[:], pe[:, 0:Z], AF.Exp,
                                 bias=bzhi[:])
            zse = gpool.tile([Z, BL], dt.float32, tag="zse")
            nc.vector.tensor_tensor(zse[:], zsc[:], eps_t[:], OP.mult)
            # z/8 = (zz_lo/8 + bz_lo/8) + zsc * eps/8, fp8 into the stream
            z_tt = nc.vector.scalar_tensor_tensor(
                azx2[:, t + 1, 0, :],
                pe[:, Z:2 * Z], bzlo[:], zse[:], OP.add, OP.add)
            state["z_tt"] = z_tt

            # closing gate matmuls for the next step (banks f,i close here;
            # g,o close on the deferred h j1 matmuls next body)
            if t + 1 < t_steps:
                for n in range(4):
                    az_mm(t + 1, n, stop=(n < 2))

            # decoder elementwise last: stays behind this step's chain
            if t >= 4:
                decoder_piece_ew(t // 4 - 1, t % 4)

        last = t_steps // 4 - 1
        decoder_ms(last - 1)
        for piece in range(4):
            decoder_piece_mm(last, piece)
            decoder_piece_ew(last, piece)
        decoder_ms(last)

    _split_multi_waits(nc)
    return nc


def _prep_host(inputs, t_steps):
    """Host-side weight/data prep -> per-core in_maps."""
    f32 = np.float32
    x, a = inputs["x"], inputs["a"]
    W_ih, W_hh = f32(inputs["W_ih"]), f32(inputs["W_hh"])
    b_g = f32(inputs["b_ih"]) + f32(inputs["b_hh"])

    # reorder gates (i,f,g,o) -> (f,i,g,o); scale f,i,o rows (and bias) by
    # 0.5 (g keeps scale 1: it gets a plain tanh)
    perm = np.concatenate([np.arange(H, 2 * H), np.arange(0, H),
                           np.arange(2 * H, 3 * H), np.arange(3 * H, 4 * H)])
    sc = np.ones(4 * H, f32)
    sc[:2 * H] = 0.5
    sc[3 * H:] = 0.5
    W_ih_r = W_ih[perm] * sc[:, None]
    W_hh_r = W_hh[perm] * sc[:, None]
    b_r = b_g[perm] * sc

    # the whole gate pre-activation is scaled by 64 (descaled in the tanh
    # via ACT scale); z is carried at 1/8 scale in fp8, so its weight rows
    # get an extra 8x
    wzah = np.zeros((128, 2, 4 * H), f32)
    wzah[:, 0, :] = 512.0 * W_ih_r[:, ACT:].T                      # z rows
    wzah[:ACT, 1, :] = 64.0 * W_ih_r[:, :ACT].T                    # a rows
    wzah[ACT, 1, :] = 64.0 * b_r                                   # bias row
    wzah = wzah.reshape(128, 2 * 4 * H).astype(f8)
    wh = np.ascontiguousarray(
        (32.0 * W_hh_r).T.reshape(4, 128, 4 * H)).astype(f8)
    w1 = np.ascontiguousarray(
        (32.0 * f32(inputs["enc_w1"])).T.reshape(4, 128, 256)).astype(f8)
    b1 = np.ascontiguousarray(64.0 * f32(inputs["enc_b1"]).reshape(2, 128).T)
    w2 = np.ascontiguousarray(
        (64.0 * f32(inputs["enc_w2"])).T.reshape(2, 128, 128)).astype(f8)
    b2 = (4096.0 * f32(inputs["enc_b2"])).reshape(128, 1)
    # zz consumes e2' = 4096*e2 -> weights /4096; lo half additionally /8
    enc_wz = f32(inputs["enc_wz"])
    wzz = np.concatenate([enc_wz[Z:, :] / 4096.0,
                          enc_wz[:Z, :] / (8.0 * 4096.0)], axis=0)
    wzz = np.ascontiguousarray(wzz.T).astype(bf16)                 # [128, 256]
    bzlo = (f32(inputs["enc_bz"])[:Z] / 8.0).reshape(Z, 1)
    bzhi = (1.0 + f32(inputs["enc_bz"])[Z:]).reshape(128, 1)

    dec_w1 = f32(inputs["dec_w1"])
    # z is carried at 1/8 scale -> 8x on its weight rows; obs part raw
    dw1z = np.ascontiguousarray(
        8.0 * dec_w1[:, STATE:].T).astype(f8)                      # [128, 64]
    dw1o = np.ascontiguousarray(
        np.concatenate([dec_w1[:, :STATE],
                        f32(inputs["dec_b1"])[:, None]],
                       axis=1).T).astype(bf16)                     # [22, 64]
    dw2 = np.ascontiguousarray(f32(inputs["dec_w2"]).T).astype(bf16)
    db2 = f32(inputs["dec_b2"]).reshape(64, 1)
    dw3 = np.ascontiguousarray(f32(inputs["dec_w3"]).T).astype(bf16)
    db3 = f32(inputs["dec_b3"]).reshape(32, 1)
    dw4 = np.ascontiguousarray(f32(inputs["dec_w4"]).T).astype(bf16)
    db4 = f32(inputs["dec_b4"]).reshape(16, 1)
    msw = np.ascontiguousarray(np.concatenate(
        [f32(inputs["mu_w"]), f32(inputs["sig_w"])], axis=0).T).astype(bf16)
    msb = np.concatenate(
        [f32(inputs["mu_b"]), 1.0 + f32(inputs["sig_b"])]).reshape(
        2 * STATE, 1)

    # eps via jax CPU (exact reference PRNG); carried at 1/8 scale
    import jax
    with jax.default_device(jax.devices("cpu")[0]):
        eps = np.asarray(jax.random.normal(
            jax.random.key(42), (T, B, Z), dtype=jax.numpy.float32))

    shared = dict(wzah=wzah, wh=wh, w1=w1, b1=b1, w2=w2, b2=b2, wzz=wzz,
                  bzlo=bzlo, bzhi=bzhi, dw1z=dw1z, dw1o=dw1o, dw2=dw2,
                  db2=db2, dw3=dw3, db3=db3, dw4=dw4, db4=db4, msw=msw,
                  msb=msb)

    in_maps = []
    for ci in range(NCORES):
        bs = slice(ci * BL, (ci + 1) * BL)
        # z/a stream: [z_{-1}=0 | a_0 | z_0 | a_1 | ...]; a_t slot holds
        # [a_t^T; 1; 0...] on 128 partitions, z slots start zeroed
        azx = np.zeros((128, 2 * t_steps + 2, BL), f32)
        a_c = f32(a[bs, :t_steps, :]).transpose(1, 2, 0)           # [T,ACT,BL]
        azx[:ACT, 1:2 * t_steps:2, :] = a_c.transpose(1, 0, 2)
        azx[ACT, 1:2 * t_steps:2, :] = 1.0
        azx = np.ascontiguousarray(
            azx.reshape(128, (2 * t_steps + 2) * BL)).astype(f8)
        eps_c = np.ascontiguousarray(
            (eps[:t_steps, bs, :] / 8.0).transpose(0, 2, 1)).astype(bf16)
        obs_c = f32(x[bs, 0, :]).T                                 # [21, BL]
        obs_rep = np.concatenate(
            [np.tile(obs_c, (1, 512 // BL)),
             np.ones((1, 512), f32)], axis=0).astype(bf16)
        m = dict(shared)
        m.update(azx=azx, eps=eps_c, obs_rep=obs_rep)
        in_maps.append(m)
    return in_maps


def _run(inputs, t_steps=T):
    from concourse.bass_utils import run_bass_kernel_spmd

    key = ("nc", t_steps)
    if key not in _CACHE:
        _CACHE[key] = _build_nc(t_steps)
    nc = _CACHE[key]
    in_maps = _prep_host(inputs, t_steps)
    res = run_bass_kernel_spmd(nc, in_maps, list(range(NCORES)),
                               trace=False)
    return res.results


def kernel(**inputs):
    t_steps = T
    results = _run(inputs, t_steps)

    y = np.float32(inputs["y"])
    su2 = 0.0
    ss = 0.0
    sabs = 0.0
    ssd = 0.0
    n_el = NCORES * STATE * t_steps * BL
    for ci in range(NCORES):
        bs = slice(ci * BL, (ci + 1) * BL)
        ms = results[ci]["ms_out"].astype(np.float64)     # [42, T*BL]
        mu = ms[:STATE]
        s = ms[STATE:]                                    # log(sigma)
        y_c = y[bs, :t_steps, :].transpose(2, 1, 0).reshape(
            STATE, t_steps * BL).astype(np.float64)
        sd = np.exp(s)
        u = (y_c - mu) / sd
        su2 += (u * u).sum()
        ss += s.sum()
        sabs += np.abs(mu - y_c).sum()
        ssd += sd.sum()
    n_tb = NCORES * t_steps * BL
    out1 = (0.5 * su2 + ss) / n_tb + STATE * LOG_SQRT_2PI
    out2 = sabs / n_el
    out3 = ssd / n_el
    return (np.float32(out1), np.float32(out2), np.float32(out3))


if __name__ == "__main__":
    import jax
    with jax.default_device(jax.devices("cpu")[0]):
        import reference as R
        inputs = {k: np.asarray(v) for k, v in R.setup_inputs().items()}
    out = kernel(**inputs)
    print("kernel:", [float(o) for o in out])


# revision 34
# speedup vs baseline: 1.3727x; 1.0709x over previous
"""Trainium2 Bass kernel for nn_MDNSeqModel: LSTM + encoder recurrence with
MDN decoder, data-parallel over batch across 8 NeuronCores.

Batch 1024 is sharded 8 ways -> 128 batch columns per core. Gate matmuls run
batch-major (activations stationary, weights moving in 512-wide windows).
Per bank the accumulation is 3 fp8 DoubleRow matmuls: h-chunk pairs (0,1),
(2,3) and the fused (z,a) pair -- z is stored fp8 at 1/8 scale interleaved
with the (padded) a_t slice in one stream tile so [z_{t-1} | a_t] is a
single DR stationary AP. Gate PSUM lives in two 2-bank tiles so the four
tanh ACT instructions collapse to two ([f,i] and [g,o]); tanh/exp/relu/copy
share one ACT table set (sigmoid = 0.5+0.5*tanh(x/2), 0.5 pre-folded into
f/i/o weight rows). Elementwise state (tanh outs, c, h) is bf16 for DVE
throughput. State carried as cD = 2c and h2 = 2h. h2 is transposed to
feature-major on the PE; psum->sbuf copies split ACT/DVE. The decoder is
interleaved into the recurrence (one 512-column chunk every 4 steps,
emitted between the h-part and the transposes so its matmuls fill the PE
during the tanh/DVE phase); mu/sigma heads run as one stacked [16,42]
matmul. Final log-prob / mean reductions run on the host in float64.
"""
import os

import numpy as np
import ml_dtypes

STATE, ACT, Z, H = 21, 8, 128, 512
B, T = 1024, 128
NCORES = 8
BL = B // NCORES          # batch per core (free dim)
LOG_SQRT_2PI = 0.9189385332046727

bf16 = ml_dtypes.bfloat16
f8 = ml_dtypes.float8_e4m3

_CACHE = {}


def _split_multi_waits(nc, max_waits=1):
    """This walrus build rejects instructions carrying more than one sync-wait
    command; Tile's semaphore pass emits up to ~4 per instruction. Hoist the
    extras onto single-wait NOPs inserted just before, on the same engine
    (each engine executes its own stream in program order, so the semantics
    are identical)."""
    import concourse.mybir as mybir

    n_nops = 0
    for f in nc.m.functions:
        for bb in f.blocks:
            insts = bb.instructions
            out = []
            changed = False
            for ins in insts:
                si = ins.sync_info
                waits = list(si.on_wait) if si is not None else []
                if len(waits) > max_waits:
                    changed = True
                    extra = waits[:-max_waits]
                    for k, w in enumerate(extra):
                        nop = mybir.InstNoOp(
                            name=f"{ins.name}-wsplit{k}", engine=ins.engine)
                        nop.sync_info = mybir.SyncInfo(
                            on_update=[], on_wait=[w])
                        out.append(nop)
                        n_nops += 1
                    while len(si.on_wait) > max_waits:
                        si.on_wait.pop(0)
                out.append(ins)
            if changed:
                bb.instructions = out
    return n_nops


def _build_nc(t_steps):
    """Build the Bass module (same NEFF for all cores; SPMD over in_maps)."""
    import contextlib

    import concourse.bass as bass
    import concourse.mybir as mybir
    import concourse.tile as tile

    dt = mybir.dt
    AF = mybir.ActivationFunctionType
    OP = mybir.AluOpType
    DR = mybir.MatmulPerfMode.DoubleRow
    NT = t_steps * BL       # decoder free length

    nc = bass.Bass()
    P = nc.declare_dram_parameter

    # ---- inputs (per-core, host-prepped) ----
    wzah_d = P("wzah", [128, 2 * 4 * H], dt.float8e4, isOutput=False)
    wh_d = P("wh", [4, 128, 4 * H], dt.float8e4, isOutput=False)   # (32*W_hh)^T
    azx_d = P("azx", [128, (2 * t_steps + 2) * BL], dt.float8e4,
              isOutput=False)                                      # z/a stream
    w1_d = P("w1", [4, 128, 256], dt.float8e4, isOutput=False)     # (32*enc_w1)^T
    b1_d = P("b1", [128, 2], dt.float32, isOutput=False)
    w2_d = P("w2", [2, 128, 128], dt.float8e4, isOutput=False)     # (64*enc_w2)^T
    b2_d = P("b2", [128, 1], dt.float32, isOutput=False)           # 4096*b2
    wzz_d = P("wzz", [128, 2 * Z], dt.bfloat16, isOutput=False)    # [hi|lo]
    bzlo_d = P("bzlo", [Z, 1], dt.float32, isOutput=False)         # bz_lo/8
    bzhi_d = P("bzhi", [128, 1], dt.float32, isOutput=False)       # 1 + bz_hi
    dw1z_d = P("dw1z", [Z, 64], dt.float8e4, isOutput=False)       # 8*w
    dw1o_d = P("dw1o", [STATE + 1, 64], dt.bfloat16, isOutput=False)  # [w|b]
    dw2_d = P("dw2", [64, 64], dt.bfloat16, isOutput=False)
    db2_d = P("db2", [64, 1], dt.float32, isOutput=False)
    dw3_d = P("dw3", [64, 32], dt.bfloat16, isOutput=False)
    db3_d = P("db3", [32, 1], dt.float32, isOutput=False)
    dw4_d = P("dw4", [32, 16], dt.bfloat16, isOutput=False)
    db4_d = P("db4", [16, 1], dt.float32, isOutput=False)
    msw_d = P("msw", [16, 2 * STATE], dt.bfloat16, isOutput=False)  # [mu|sig]
    msb_d = P("msb", [2 * STATE, 1], dt.float32, isOutput=False)
    obs_d = P("obs_rep", [STATE + 1, 512], dt.bfloat16, isOutput=False)
    eps_d = P("eps", [t_steps, Z, BL], dt.bfloat16, isOutput=False)  # eps/8

    ms_o = P("ms_out", [2 * STATE, NT], dt.float32, isOutput=True)

    with tile.TileContext(nc) as tc, contextlib.ExitStack() as octx:
        wpool = octx.enter_context(tc.tile_pool(name="weights", bufs=1))
        spool = octx.enter_context(tc.tile_pool(name="state", bufs=1))
        gpool = octx.enter_context(tc.tile_pool(name="gwork", bufs=2))
        epool = octx.enter_context(tc.tile_pool(name="eps", bufs=4))
        dpool = octx.enter_context(tc.tile_pool(name="dwork", bufs=3))
        gpsum = octx.enter_context(
            tc.tile_pool(name="gpsum", bufs=1, space="PSUM"))
        epsum = octx.enter_context(
            tc.tile_pool(name="epsum", bufs=1, space="PSUM"))
        dpsum = octx.enter_context(
            tc.tile_pool(name="dpsum", bufs=3, space="PSUM"))

        # ---- load weights ----
        wzah = wpool.tile([128, 2 * 4 * H], dt.float8e4)
        wh = wpool.tile([128, 4 * 4 * H], dt.float8e4)
        w1 = wpool.tile([128, 4 * 256], dt.float8e4)
        w2 = wpool.tile([128, 2 * 128], dt.float8e4)
        wzz = wpool.tile([128, 2 * Z], dt.bfloat16)
        b1 = wpool.tile([128, 2], dt.float32)
        b2 = wpool.tile([128, 1], dt.float32)
        bzlo = wpool.tile([Z, 1], dt.float32)
        bzhi = wpool.tile([128, 1], dt.float32)
        nc.sync.dma_start(out=wzah[:], in_=wzah_d[:])
        for k in range(4):
            nc.sync.dma_start(
                out=wh[:, 2048 * k:2048 * (k + 1)], in_=wh_d[k, :, :])
            nc.sync.dma_start(
                out=w1[:, 256 * k:256 * (k + 1)], in_=w1_d[k, :, :])
        for k in range(2):
            nc.sync.dma_start(
                out=w2[:, 128 * k:128 * (k + 1)], in_=w2_d[k, :, :])
        nc.sync.dma_start(out=wzz[:], in_=wzz_d[:])
        nc.sync.dma_start(out=b1[:], in_=b1_d[:])
        nc.sync.dma_start(out=b2[:], in_=b2_d[:])
        nc.sync.dma_start(out=bzlo[:], in_=bzlo_d[:])
        nc.sync.dma_start(out=bzhi[:], in_=bzhi_d[:])

        # z/a stream: [z_{-1}=0 | a_0 | z_0 | a_1 | ... | z_{T-1} | pad],
        # fp8, z at 1/8; pair s = [z_{s-1} | a_s]
        azx = wpool.tile([128, (2 * t_steps + 2) * BL], dt.float8e4)
        nc.sync.dma_start(out=azx[:], in_=azx_d[:])
        azx2 = azx[:].rearrange("p (s two b) -> p s two b", two=2, b=BL)
        wzahv = wzah[:].rearrange("p (two w) -> p two w", two=2)



        # decoder weights
        dw1z = wpool.tile([Z, 64], dt.float8e4)
        dw1o = wpool.tile([STATE + 1, 64], dt.bfloat16)
        dw2 = wpool.tile([64, 64], dt.bfloat16)
        dw3 = wpool.tile([64, 32], dt.bfloat16)
        dw4 = wpool.tile([32, 16], dt.bfloat16)
        msw = wpool.tile([16, 2 * STATE], dt.bfloat16)
        db2 = wpool.tile([64, 1], dt.float32)
        db3 = wpool.tile([32, 1], dt.float32)
        db4 = wpool.tile([16, 1], dt.float32)
        msb = wpool.tile([2 * STATE, 1], dt.float32)
        obs = wpool.tile([STATE + 1, 512], dt.bfloat16)
        for tdst, tsrc in [(dw1z, dw1z_d), (dw1o, dw1o_d), (dw2, dw2_d),
                           (dw3, dw3_d), (dw4, dw4_d), (msw, msw_d),
                           (db2, db2_d), (db3, db3_d), (db4, db4_d),
                           (msb, msb_d), (obs, obs_d)]:
            nc.sync.dma_start(out=tdst[:], in_=tsrc[:])

        # ---- state ----
        h2b = spool.tile([128, H], dt.bfloat16)      # 2*h, batch-major
        h2f = spool.tile([128, H], dt.float8e4)      # 2*h, feature-major fp8
        cd = spool.tile([128, H], dt.bfloat16)       # 2*c, batch-major
        ident = spool.tile([128, 128], dt.bfloat16)
        nc.vector.memset(h2f[:], 0.0)
        nc.vector.memset(cd[:], 0.0)
        from concourse.masks import make_identity
        make_identity(nc, ident[:])

        # gates PSUM: one 2-bank tile (f,i) so their tanh is a single
        # [128,1024] ACT instruction, plus single-bank tiles for g and o so
        # tanh_g / tanh_o only wait on their own bank's writers (Tile RAW
        # tracking is per-tile)
        pg01 = gpsum.tile([128, 1024], dt.float32, tag="pg01", name="pg01")
        pgg = gpsum.tile([128, 512], dt.float32, tag="pgg", name="pgg")
        pgo = gpsum.tile([128, 512], dt.float32, tag="pgo", name="pgo")

        def gbank(n):
            """psum region of gate bank n (0..3 = f,i,g,o)."""
            if n < 2:
                return pg01[:, 512 * n:512 * (n + 1)]
            return (pgg if n == 2 else pgo)[:]

        dec_state = {}

        from bass_rust import add_dep_helper as _add_dep
        state = {}

        def _pin_dve(op):
            if "z_tt" in state:
                _add_dep(op.ins, state["z_tt"].ins, sync=False,
                         reason="decoder DVE stays behind the z chain")
            return op

        def _pin_mm(op):
            if "zz_mm" in state:
                _add_dep(op.ins, state["zz_mm"].ins, sync=False,
                         reason="decoder MM stays behind the encoder chain")
            return op

        def decoder_piece_mm(cidx, piece):
            """Decoder matmuls: emitted right after the h-part so they fill
            the PE while ACT/DVE run the gate tail. Piece 0 also carries the
            mu/sigma head matmul of the previous chunk (5 stages, 4 slots)."""
            if piece == 0:
                if cidx >= 1:
                    pm = dpsum.tile([2 * STATE, 512], dt.float32, tag="dscr")
                    _pin_mm(nc.tensor.matmul(
                        pm[:], msw[:], dec_state["d4"][:],
                        start=True, stop=True))
                    dec_state["pm"] = pm
                p1 = dpsum.tile([64, 512], dt.float32, tag="dscr")
                zblk = azx2[:, 4 * cidx + 1:4 * cidx + 5, 0, :]
                _pin_mm(nc.tensor.matmul(p1[:], dw1z[:], zblk,
                                         start=True, stop=False))
                nc.tensor.matmul(p1[:], dw1o[:], obs[:],
                                 start=False, stop=True)
                dec_state["p"] = p1
            elif piece == 1:
                p2 = dpsum.tile([64, 512], dt.float32, tag="dscr")
                _pin_mm(nc.tensor.matmul(p2[:], dw2[:], dec_state["d1"][:],
                                         start=True, stop=True))
                dec_state["p"] = p2
            elif piece == 2:
                p3 = dpsum.tile([32, 512], dt.float32, tag="dscr")
                _pin_mm(nc.tensor.matmul(p3[:], dw3[:], dec_state["d2"][:],
                                         start=True, stop=True))
                dec_state["p"] = p3
            else:
                p4 = dpsum.tile([16, 512], dt.float32, tag="dscr")
                _pin_mm(nc.tensor.matmul(p4[:], dw4[:], dec_state["d3"][:],
                                         start=True, stop=True))
                dec_state["p"] = p4

        def decoder_piece_ew(cidx, piece):
            """Decoder elementwise: emitted after z_tt so the DVE in-order
            stream never blocks the gate tail behind a decoder matmul."""
            p = dec_state["p"]
            if piece == 0:
                if cidx >= 1:
                    cs = slice(512 * (cidx - 1), 512 * cidx)
                    ms_sb = dpool.tile([2 * STATE, 512], dt.float32,
                                       tag="ms")
                    _pin_dve(nc.vector.tensor_scalar_add(
                        ms_sb[:], dec_state["pm"][:], msb[:]))
                    nc.sync.dma_start(out=ms_o[:, cs], in_=ms_sb[:])
                d1 = dpool.tile([64, 512], dt.bfloat16, tag="d1")
                _pin_dve(nc.vector.tensor_scalar(
                    d1[:], p[:], 0.0, None, OP.max))
                dec_state["d1"] = d1
            elif piece == 1:
                d2 = dpool.tile([64, 512], dt.bfloat16, tag="d2")
                _pin_dve(nc.vector.tensor_scalar(
                    d2[:], p[:], db2[:], 0.0, OP.add, OP.max))
                dec_state["d2"] = d2
            elif piece == 2:
                d3 = dpool.tile([32, 512], dt.bfloat16, tag="d3")
                _pin_dve(nc.vector.tensor_scalar(
                    d3[:], p[:], db3[:], 0.0, OP.add, OP.max))
                dec_state["d3"] = d3
            else:
                d4 = dpool.tile([16, 512], dt.bfloat16, tag="d4")
                _pin_dve(nc.vector.tensor_scalar(
                    d4[:], p[:], db4[:], 0.0, OP.add, OP.max))
                dec_state["d4"] = d4

        def decoder_ms(cidx):
            pm = dpsum.tile([2 * STATE, 512], dt.float32, tag="dscr")
            nc.tensor.matmul(pm[:], msw[:], dec_state["d4"][:],
                             start=True, stop=True)
            ms_sb = dpool.tile([2 * STATE, 512], dt.float32, tag="ms")
            nc.vector.tensor_scalar_add(ms_sb[:], pm[:], msb[:])
            nc.sync.dma_start(out=ms_o[:, 512 * cidx:512 * (cidx + 1)],
                              in_=ms_sb[:])

        whv = wh[:].rearrange("p (k w) -> p k w", k=4)
        w1v = w1[:].rearrange("p (k w) -> p k w", k=4)

        def h_mm(t, j, n, start=False, stop=False):
            stat = h2f[:, 256 * j:256 * (j + 1)].rearrange(
                "p (two b) -> p two b", two=2)
            return nc.tensor.matmul(
                gbank(n), stat,
                whv[:, 2 * j:2 * (j + 1), 512 * n:512 * (n + 1)],
                start=start, stop=stop,
                perf_mode=DR)

        def az_mm(t, n, stop=False):
            # fused (z_{t-1}, a_t) DoubleRow accumulation
            return nc.tensor.matmul(
                gbank(n), azx2[:, t, :, :],
                wzahv[:, :, 512 * n:512 * (n + 1)],
                start=False, stop=stop,
                perf_mode=DR)

        # ---- recurrence ----
        # Per-bank accumulation groups: h j0 opens (start); banks f,i close
        # on their az matmul, banks g,o close on the h j1 matmul which is
        # DEFERRED into the next body so it streams during tanh(f,i) and
        # keeps the PE from going cold (p-state) in the gate tail.
        # gates(0): h2f and z_{-1} are zero
        for n in range(4):
            h_mm(0, 0, n, start=True)
        h_mm(0, 1, 0)
        h_mm(0, 1, 1)
        for n in range(4):
            az_mm(0, n, stop=(n < 2))
        for t in range(t_steps):
            eps_t = epool.tile([Z, BL], dt.bfloat16, tag="eps")
            nc.sync.dma_start(out=eps_t[:], in_=eps_d[t, :, :])

            # tanh over (f,i) merged; banks g,o close during it (deferred
            # h j1 matmuls keep the PE busy through the ACT phase)
            tg = gpool.tile([128, 4 * H], dt.float32, tag="tanh_g")
            nc.scalar.activation(tg[:, 0:1024], pg01[:], AF.Tanh,
                                 scale=1.0 / 64.0)
            h_mm(t, 1, 2, stop=True)
            h_mm(t, 1, 3, stop=True)
            nc.scalar.activation(tg[:, 1024:1536], pgg[:],
                                 AF.Tanh, scale=1.0 / 64.0)
            nc.scalar.activation(tg[:, 1536:2048], pgo[:],
                                 AF.Tanh, scale=1.0 / 64.0)

            t_f = tg[:, 0:512]
            t_i = tg[:, 512:1024]
            t_g = tg[:, 1024:1536]
            t_o = tg[:, 1536:2048]

            tmp1 = gpool.tile([128, H], dt.float32, tag="tmp1")
            tmp2 = gpool.tile([128, H], dt.float32, tag="tmp2")
            tcn = gpool.tile([128, H], dt.float32, tag="tanh_c")
            # tmp1 = (1+tanh(f/2)) * cD ; then per 256-half:
            # tmp2 = (1+tanh(i/2)) * g ; cD = 0.5*tmp1 + tmp2 (= 2*c_new)
            nc.vector.scalar_tensor_tensor(
                tmp1[:], t_f, 1.0, cd[:], OP.add, OP.mult)
            for hh in range(2):
                hs = slice(256 * hh, 256 * (hh + 1))
                nc.vector.scalar_tensor_tensor(
                    tmp2[:, hs], t_i[:, hs], 1.0, t_g[:, hs],
                    OP.add, OP.mult)
                nc.vector.scalar_tensor_tensor(
                    cd[:, hs], tmp1[:, hs], 0.5, tmp2[:, hs],
                    OP.mult, OP.add)
            # tc = tanh(0.5*cD); h2 = (1+tanh(o/2)) * tc, in halves so the
            # transposes / h-part start on the first half early
            for hh in range(2):
                hs = slice(256 * hh, 256 * (hh + 1))
                nc.scalar.activation(tcn[:, hs], cd[:, hs], AF.Tanh,
                                     scale=0.5)
                nc.vector.scalar_tensor_tensor(
                    h2b[:, hs], t_o[:, hs], 1.0, tcn[:, hs],
                    OP.add, OP.mult)

            # transpose h2 batch-major -> feature-major on the PE;
            # psum->sbuf copies split between ACT and DVE
            for k in range(4):
                ptr = dpsum.tile([128, 128], dt.bfloat16, tag="dscr")
                nc.tensor.transpose(
                    ptr[:], h2b[:, 128 * k:128 * (k + 1)], ident[:])
                dst = h2f[:, 128 * k:128 * (k + 1)]
                if k % 2 == 0:
                    nc.scalar.copy(dst, ptr[:])
                else:
                    nc.vector.tensor_copy(dst, ptr[:])

            # encoder (feature-major): e1/e2/zz share one PSUM bank; e1 via
            # fp8 DoubleRow; h(t+1) j0 matmuls interleave into the z-path's
            # PE wait slots so the PE never idles long enough to re-throttle
            pe = epsum.tile([128, 384], dt.float32, tag="enc")
            for m in range(2):
                out = pe[:, 128 * m:128 * (m + 1)]
                for j in range(2):
                    state["e1_mm"] = nc.tensor.matmul(
                        out,
                        w1v[:, 2 * j:2 * (j + 1), 128 * m:128 * (m + 1)],
                        h2f[:, 256 * j:256 * (j + 1)].rearrange(
                            "p (two b) -> p two b", two=2),
                        start=(j == 0), stop=(j == 1),
                        perf_mode=DR)
            # e1 stored as 64*e1 in fp8 (descale folded into enc_w2)
            e1 = gpool.tile([128, 256], dt.float8e4, tag="e1")
            nc.scalar.activation(e1[:, 0:128], pe[:, 0:128], AF.Relu,
                                 bias=b1[:, 0:1])
            nc.vector.tensor_scalar(
                e1[:, 128:256], pe[:, 128:256],
                b1[:, 1:2], 0.0, OP.add, OP.max)
            if t + 1 < t_steps:
                for n in range(4):
                    mm = h_mm(t + 1, 0, n, start=True)
                    _add_dep(mm.ins, state["e1_mm"].ins, sync=False,
                             reason="z-path e1 streams before the h-block")
            # e2' = max(psum + 4096*b2, 0) = 4096*e2 (descale in wzz)
            out = pe[:, 256:384]
            state["e2_mm"] = nc.tensor.matmul(
                out, w2[:].rearrange("p (k m) -> p k m", k=2),
                e1[:].rearrange("p (two b) -> p two b", two=2),
                start=True, stop=True,
                perf_mode=DR)
            e2 = gpool.tile([128, 128], dt.bfloat16, tag="e2")
            nc.vector.tensor_scalar(
                e2[:], out, b2[:], 0.0, OP.add, OP.max)
            # zz = WZ @ e2' (weights pre-descaled); hi first: the exp->zse
            # path is the longer consumer
            zz_mm = nc.tensor.matmul(pe[:, 0:Z], wzz[:, 0:Z], e2[:],
                                     start=True, stop=True)
            zz_lo = nc.tensor.matmul(pe[:, Z:2 * Z], wzz[:, Z:2 * Z], e2[:],
                                     start=True, stop=True)
            state["zz_mm"] = zz_mm
            if t + 1 < t_steps:
                for n in (0, 1):
                    mm = h_mm(t + 1, 1, n)
                    _add_dep(mm.ins, zz_lo.ins, sync=False,
                             reason="zz streams before the h j1 block")
            if t >= 4:
                decoder_piece_mm(t // 4 - 1, t % 4)
            # z_scale = exp(zz_hi + (1+bz_hi))
            zsc = gpool.tile([Z, BL], dt.float32, tag="zsc")
            nc.scalar.activation(zsc[:], pe[:, 0:Z], AF.Exp,
                                 bias=bzhi[:])
            zse = gpool.tile([Z, BL], dt.float32, tag="zse")
            nc.vector.tensor_tensor(zse[:], zsc[:], eps_t[:], OP.mult)
            # z/8 = (zz_lo/8 + bz_lo/8) + zsc * eps/8, fp8 into the stream
            z_tt = nc.vector.scalar_tensor_tensor(
                azx2[:, t + 1, 0, :],
                pe[:, Z:2 * Z], bzlo[:], zse[:], OP.add, OP.add)
            state["z_tt"] = z_tt

            # closing gate matmuls for the next step (banks f,i close here;
            # g,o close on the deferred h j1 matmuls next body)
            if t + 1 < t_steps:
                for n in range(4):
                    az_mm(t + 1, n, stop=(n < 2))

            # decoder elementwise last: stays behind this step's chain
            if t >= 4:
                decoder_piece_ew(t // 4 - 1, t % 4)

        last = t_steps // 4 - 1
        decoder_ms(last - 1)
        for piece in range(4):
            decoder_piece_mm(last, piece)
            decoder_piece_ew(last, piece)
        decoder_ms(last)

    _split_multi_waits(nc)
    return nc


def _prep_host(inputs, t_steps):
    """Host-side weight/data prep -> per-core in_maps."""
    f32 = np.float32
    x, a = inputs["x"], inputs["a"]
    W_ih, W_hh = f32(inputs["W_ih"]), f32(inputs["W_hh"])
    b_g = f32(inputs["b_ih"]) + f32(inputs["b_hh"])

    # reorder gates (i,f,g,o) -> (f,i,g,o); scale f,i,o rows (and bias) by
    # 0.5 (g keeps scale 1: it gets a plain tanh)
    perm = np.concatenate([np.arange(H, 2 * H), np.arange(0, H),
                           np.arange(2 * H, 3 * H), np.arange(3 * H, 4 * H)])
    sc = np.ones(4 * H, f32)
    sc[:2 * H] = 0.5
    sc[3 * H:] = 0.5
    W_ih_r = W_ih[perm] * sc[:, None]
    W_hh_r = W_hh[perm] * sc[:, None]
    b_r = b_g[perm] * sc

    # the whole gate pre-activation is scaled by 64 (descaled in the tanh
    # via ACT scale); z is carried at 1/8 scale in fp8, so its weight rows
    # get an extra 8x
    wzah = np.zeros((128, 2, 4 * H), f32)
    wzah[:, 0, :] = 512.0 * W_ih_r[:, ACT:].T                      # z rows
    wzah[:ACT, 1, :] = 64.0 * W_ih_r[:, :ACT].T                    # a rows
    wzah[ACT, 1, :] = 64.0 * b_r                                   # bias row
    wzah = wzah.reshape(128, 2 * 4 * H).astype(f8)
    wh = np.ascontiguousarray(
        (32.0 * W_hh_r).T.reshape(4, 128, 4 * H)).astype(f8)
    w1 = np.ascontiguousarray(
        (32.0 * f32(inputs["enc_w1"])).T.reshape(4, 128, 256)).astype(f8)
    b1 = np.ascontiguousarray(64.0 * f32(inputs["enc_b1"]).reshape(2, 128).T)
    w2 = np.ascontiguousarray(
        (64.0 * f32(inputs["enc_w2"])).T.reshape(2, 128, 128)).astype(f8)
    b2 = (4096.0 * f32(inputs["enc_b2"])).reshape(128, 1)
    # zz consumes e2' = 4096*e2 -> weights /4096; lo half additionally /8
    enc_wz = f32(inputs["enc_wz"])
    wzz = np.concatenate([enc_wz[Z:, :] / 4096.0,
                          enc_wz[:Z, :] / (8.0 * 4096.0)], axis=0)
    wzz = np.ascontiguousarray(wzz.T).astype(bf16)                 # [128, 256]
    bzlo = (f32(inputs["enc_bz"])[:Z] / 8.0).reshape(Z, 1)
    bzhi = (1.0 + f32(inputs["enc_bz"])[Z:]).reshape(128, 1)

    dec_w1 = f32(inputs["dec_w1"])
    # z is carried at 1/8 scale -> 8x on its weight rows; obs part raw
    dw1z = np.ascontiguousarray(
        8.0 * dec_w1[:, STATE:].T).astype(f8)                      # [128, 64]
    dw1o = np.ascontiguousarray(
        np.concatenate([dec_w1[:, :STATE],
                        f32(inputs["dec_b1"])[:, None]],
                       axis=1).T).astype(bf16)                     # [22, 64]
    dw2 = np.ascontiguousarray(f32(inputs["dec_w2"]).T).astype(bf16)
    db2 = f32(inputs["dec_b2"]).reshape(64, 1)
    dw3 = np.ascontiguousarray(f32(inputs["dec_w3"]).T).astype(bf16)
    db3 = f32(inputs["dec_b3"]).reshape(32, 1)
    dw4 = np.ascontiguousarray(f32(inputs["dec_w4"]).T).astype(bf16)
    db4 = f32(inputs["dec_b4"]).reshape(16, 1)
    msw = np.ascontiguousarray(np.concatenate(
        [f32(inputs["mu_w"]), f32(inputs["sig_w"])], axis=0).T).astype(bf16)
    msb = np.concatenate(
        [f32(inputs["mu_b"]), 1.0 + f32(inputs["sig_b"])]).reshape(
        2 * STATE, 1)

    # eps via jax CPU (exact reference PRNG); carried at 1/8 scale
    import jax
    with jax.default_device(jax.devices("cpu")[0]):
        eps = np.asarray(jax.random.normal(
            jax.random.key(42), (T, B, Z), dtype=jax.numpy.float32))

    shared = dict(wzah=wzah, wh=wh, w1=w1, b1=b1, w2=w2, b2=b2, wzz=wzz,
                  bzlo=bzlo, bzhi=bzhi, dw1z=dw1z, dw1o=dw1o, dw2=dw2,
                  db2=db2, dw3=dw3, db3=db3, dw4=dw4, db4=db4, msw=msw,
                  msb=msb)

    in_maps = []
    for ci in range(NCORES):
        bs = slice(ci * BL, (ci + 1) * BL)
        # z/a stream: [z_{-1}=0 | a_0 | z_0 | a_1 | ...]; a_t slot holds
        # [a_t^T; 1; 0...] on 128 partitions, z slots start zeroed
        azx = np.zeros((128, 2 * t_steps + 2, BL), f32)
        a_c = f32(a[bs, :t_steps, :]).transpose(1, 2, 0)           # [T,ACT,BL]
        azx[:ACT, 1:2 * t_steps:2, :] = a_c.transpose(1, 0, 2)
        azx[ACT, 1:2 * t_steps:2, :] = 1.0
        azx = np.ascontiguousarray(
            azx.reshape(128, (2 * t_steps + 2) * BL)).astype(f8)
        eps_c = np.ascontiguousarray(
            (eps[:t_steps, bs, :] / 8.0).transpose(0, 2, 1)).astype(bf16)
        obs_c = f32(x[bs, 0, :]).T                                 # [21, BL]
        obs_rep = np.concatenate(
            [np.tile(obs_c, (1, 512 // BL)),
             np.ones((1, 512), f32)], axis=0).astype(bf16)
        m = dict(shared)
        m.update(azx=azx, eps=eps_c, obs_rep=obs_rep)
        in_maps.append(m)
    return in_maps


def _run(inputs, t_steps=T):
    from concourse.bass_utils import run_bass_kernel_spmd

    key = ("nc", t_steps)
    if key not in _CACHE:
        _CACHE[key] = _build_nc(t_steps)
    nc = _CACHE[key]
    in_maps = _prep_host(inputs, t_steps)
    res = run_bass_kernel_spmd(nc, in_maps, list(range(NCORES)),
                               trace=False)
    return res.results


def kernel(**inputs):
    t_steps = T
    results = _run(inputs, t_steps)

    y = np.float32(inputs["y"])
    su2 = 0.0
    ss = 0.0
    sabs = 0.0
    ssd = 0.0
    n_el = NCORES * STATE * t_steps * BL
    for ci in range(NCORES):
        bs = slice(ci * BL, (ci + 1) * BL)
        ms = results[ci]["ms_out"].astype(np.float64)     # [42, T*BL]
        mu = ms[:STATE]
        s = ms[STATE:]                                    # log(sigma)
        y_c = y[bs, :t_steps, :].transpose(2, 1, 0).reshape(
            STATE, t_steps * BL).astype(np.float64)
        sd = np.exp(s)
        u = (y_c - mu) / sd
        su2 += (u * u).sum()
        ss += s.sum()
        sabs += np.abs(mu - y_c).sum()
        ssd += sd.sum()
    n_tb = NCORES * t_steps * BL
    out1 = (0.5 * su2 + ss) / n_tb + STATE * LOG_SQRT_2PI
    out2 = sabs / n_el
    out3 = ssd / n_el
    return (np.float32(out1), np.float32(out2), np.float32(out3))


if __name__ == "__main__":
    import jax
    with jax.default_device(jax.devices("cpu")[0]):
        import reference as R
        inputs = {k: np.asarray(v) for k, v in R.setup_inputs().items()}
    out = kernel(**inputs)
    print("kernel:", [float(o) for o in out])


# revision 38
# speedup vs baseline: 1.4761x; 1.0753x over previous
"""Trainium2 Bass kernel for nn_MDNSeqModel: LSTM + encoder recurrence with
MDN decoder, data-parallel over batch across 8 NeuronCores.

Batch 1024 is sharded 8 ways -> 128 batch columns per core. Gate matmuls run
batch-major (activations stationary, weights moving in 512-wide windows).
Per bank the accumulation is 3 fp8 DoubleRow matmuls: h-chunk pairs (0,1),
(2,3) and the fused (z,a) pair -- z is stored fp8 at 1/8 scale interleaved
with the (padded) a_t slice in one stream tile so [z_{t-1} | a_t] is a
single DR stationary AP. Gate PSUM lives in two 2-bank tiles so the four
tanh ACT instructions collapse to two ([f,i] and [g,o]); tanh/exp/relu/copy
share one ACT table set (sigmoid = 0.5+0.5*tanh(x/2), 0.5 pre-folded into
f/i/o weight rows). Elementwise state (tanh outs, c, h) is bf16 for DVE
throughput. State carried as cD = 2c and h2 = 2h. h2 is transposed to
feature-major on the PE; psum->sbuf copies split ACT/DVE. The decoder is
interleaved into the recurrence (one 512-column chunk every 4 steps,
emitted between the h-part and the transposes so its matmuls fill the PE
during the tanh/DVE phase); mu/sigma heads run as one stacked [16,42]
matmul. Final log-prob / mean reductions run on the host in float64.
"""
import os

import numpy as np
import ml_dtypes

STATE, ACT, Z, H = 21, 8, 128, 512
B, T = 1024, 128
NCORES = 8
BL = B // NCORES          # batch per core (free dim)
LOG_SQRT_2PI = 0.9189385332046727

bf16 = ml_dtypes.bfloat16
f8 = ml_dtypes.float8_e4m3

_CACHE = {}


def _split_multi_waits(nc, max_waits=1):
    """This walrus build rejects instructions carrying more than one sync-wait
    command; Tile's semaphore pass emits up to ~4 per instruction. Hoist the
    extras onto single-wait NOPs inserted just before, on the same engine
    (each engine executes its own stream in program order, so the semantics
    are identical)."""
    import concourse.mybir as mybir

    n_nops = 0
    for f in nc.m.functions:
        for bb in f.blocks:
            insts = bb.instructions
            out = []
            changed = False
            for ins in insts:
                si = ins.sync_info
                waits = list(si.on_wait) if si is not None else []
                if len(waits) > max_waits:
                    changed = True
                    extra = waits[:-max_waits]
                    for k, w in enumerate(extra):
                        nop = mybir.InstNoOp(
                            name=f"{ins.name}-wsplit{k}", engine=ins.engine)
                        nop.sync_info = mybir.SyncInfo(
                            on_update=[], on_wait=[w])
                        out.append(nop)
                        n_nops += 1
                    while len(si.on_wait) > max_waits:
                        si.on_wait.pop(0)
                out.append(ins)
            if changed:
                bb.instructions = out
    return n_nops


def _build_nc(t_steps):
    """Build the Bass module (same NEFF for all cores; SPMD over in_maps)."""
    import contextlib

    import concourse.bass as bass
    import concourse.mybir as mybir
    import concourse.tile as tile

    dt = mybir.dt
    AF = mybir.ActivationFunctionType
    OP = mybir.AluOpType
    DR = mybir.MatmulPerfMode.DoubleRow
    NT = t_steps * BL       # decoder free length

    nc = bass.Bass()
    P = nc.declare_dram_parameter

    # ---- inputs (per-core, host-prepped) ----
    wzah_d = P("wzah", [128, 2 * 4 * H], dt.float8e4, isOutput=False)
    wh_d = P("wh", [4, 128, 4 * H], dt.float8e4, isOutput=False)   # (32*W_hh)^T
    azx_d = P("azx", [128, (2 * t_steps + 2) * BL], dt.float8e4,
              isOutput=False)                                      # z/a stream
    w1_d = P("w1", [4, 128, 256], dt.float8e4, isOutput=False)     # (32*enc_w1)^T
    b1_d = P("b1", [128, 2], dt.float32, isOutput=False)
    w2_d = P("w2", [2, 128, 128], dt.float8e4, isOutput=False)     # (64*enc_w2)^T
    b2_d = P("b2", [128, 1], dt.float32, isOutput=False)           # 4096*b2
    wzz_d = P("wzz", [128, 2 * Z], dt.bfloat16, isOutput=False)    # [hi|lo]
    bzlo_d = P("bzlo", [Z, 1], dt.float32, isOutput=False)         # bz_lo/8
    bzhi_d = P("bzhi", [128, 1], dt.float32, isOutput=False)       # 1 + bz_hi
    dw1z_d = P("dw1z", [Z, 64], dt.float8e4, isOutput=False)       # 8*w
    dw1o_d = P("dw1o", [STATE + 1, 64], dt.bfloat16, isOutput=False)  # [w|b]
    dw2_d = P("dw2", [64, 64], dt.bfloat16, isOutput=False)
    db2_d = P("db2", [64, 1], dt.float32, isOutput=False)
    dw3_d = P("dw3", [64, 32], dt.bfloat16, isOutput=False)
    db3_d = P("db3", [32, 1], dt.float32, isOutput=False)
    dw4_d = P("dw4", [32, 16], dt.bfloat16, isOutput=False)
    db4_d = P("db4", [16, 1], dt.float32, isOutput=False)
    msw_d = P("msw", [16, 2 * STATE], dt.bfloat16, isOutput=False)  # [mu|sig]
    msb_d = P("msb", [2 * STATE, 1], dt.float32, isOutput=False)
    obs_d = P("obs_rep", [STATE + 1, 512], dt.bfloat16, isOutput=False)
    eps_d = P("eps", [t_steps, Z, BL], dt.bfloat16, isOutput=False)  # eps/8

    ms_o = P("ms_out", [2 * STATE, NT], dt.float32, isOutput=True)

    with tile.TileContext(nc) as tc, contextlib.ExitStack() as octx:
        wpool = octx.enter_context(tc.tile_pool(name="weights", bufs=1))
        spool = octx.enter_context(tc.tile_pool(name="state", bufs=1))
        gpool = octx.enter_context(tc.tile_pool(name="gwork", bufs=2))
        epool = octx.enter_context(tc.tile_pool(name="eps", bufs=4))
        dpool = octx.enter_context(tc.tile_pool(name="dwork", bufs=3))
        gpsum = octx.enter_context(
            tc.tile_pool(name="gpsum", bufs=1, space="PSUM"))
        epsum = octx.enter_context(
            tc.tile_pool(name="epsum", bufs=1, space="PSUM"))
        dpsum = octx.enter_context(
            tc.tile_pool(name="dpsum", bufs=3, space="PSUM"))

        # ---- load weights ----
        wzah = wpool.tile([128, 2 * 4 * H], dt.float8e4)
        wh = wpool.tile([128, 4 * 4 * H], dt.float8e4)
        w1 = wpool.tile([128, 4 * 256], dt.float8e4)
        w2 = wpool.tile([128, 2 * 128], dt.float8e4)
        wzz = wpool.tile([128, 2 * Z], dt.bfloat16)
        b1 = wpool.tile([128, 2], dt.float32)
        b2 = wpool.tile([128, 1], dt.float32)
        bzlo = wpool.tile([Z, 1], dt.float32)
        bzhi = wpool.tile([128, 1], dt.float32)
        nc.sync.dma_start(out=wzah[:], in_=wzah_d[:])
        for k in range(4):
            nc.sync.dma_start(
                out=wh[:, 2048 * k:2048 * (k + 1)], in_=wh_d[k, :, :])
            nc.sync.dma_start(
                out=w1[:, 256 * k:256 * (k + 1)], in_=w1_d[k, :, :])
        for k in range(2):
            nc.sync.dma_start(
                out=w2[:, 128 * k:128 * (k + 1)], in_=w2_d[k, :, :])
        nc.sync.dma_start(out=wzz[:], in_=wzz_d[:])
        nc.sync.dma_start(out=b1[:], in_=b1_d[:])
        nc.sync.dma_start(out=b2[:], in_=b2_d[:])
        nc.sync.dma_start(out=bzlo[:], in_=bzlo_d[:])
        nc.sync.dma_start(out=bzhi[:], in_=bzhi_d[:])

        # z/a stream: [z_{-1}=0 | a_0 | z_0 | a_1 | ... | z_{T-1} | pad],
        # fp8, z at 1/8; pair s = [z_{s-1} | a_s]
        azx = wpool.tile([128, (2 * t_steps + 2) * BL], dt.float8e4)
        nc.sync.dma_start(out=azx[:], in_=azx_d[:])
        azx2 = azx[:].rearrange("p (s two b) -> p s two b", two=2, b=BL)
        wzahv = wzah[:].rearrange("p (two w) -> p two w", two=2)



        # decoder weights
        dw1z = wpool.tile([Z, 64], dt.float8e4)
        dw1o = wpool.tile([STATE + 1, 64], dt.bfloat16)
        dw2 = wpool.tile([64, 64], dt.bfloat16)
        dw3 = wpool.tile([64, 32], dt.bfloat16)
        dw4 = wpool.tile([32, 16], dt.bfloat16)
        msw = wpool.tile([16, 2 * STATE], dt.bfloat16)
        db2 = wpool.tile([64, 1], dt.float32)
        db3 = wpool.tile([32, 1], dt.float32)
        db4 = wpool.tile([16, 1], dt.float32)
        msb = wpool.tile([2 * STATE, 1], dt.float32)
        obs = wpool.tile([STATE + 1, 512], dt.bfloat16)
        for tdst, tsrc in [(dw1z, dw1z_d), (dw1o, dw1o_d), (dw2, dw2_d),
                           (dw3, dw3_d), (dw4, dw4_d), (msw, msw_d),
                           (db2, db2_d), (db3, db3_d), (db4, db4_d),
                           (msb, msb_d), (obs, obs_d)]:
            nc.sync.dma_start(out=tdst[:], in_=tsrc[:])

        # ---- state ----
        h2b = spool.tile([128, H], dt.bfloat16)      # 2*h, batch-major
        h2f = spool.tile([128, H], dt.float8e4)      # 2*h, feature-major fp8
        cd = spool.tile([128, H], dt.bfloat16)       # 2*c, batch-major
        ident = spool.tile([128, 128], dt.bfloat16)
        nc.vector.memset(h2f[:], 0.0)
        nc.vector.memset(cd[:], 0.0)
        from concourse.masks import make_identity
        make_identity(nc, ident[:])

        # gates PSUM: one 2-bank tile (f,i) so their tanh is a single
        # [128,1024] ACT instruction, plus single-bank tiles for g and o so
        # tanh_g / tanh_o only wait on their own bank's writers (Tile RAW
        # tracking is per-tile)
        pg01 = gpsum.tile([128, 1024], dt.float32, tag="pg01", name="pg01")
        pgg = gpsum.tile([128, 512], dt.float32, tag="pgg", name="pgg")
        pgo = gpsum.tile([128, 512], dt.float32, tag="pgo", name="pgo")

        def gbank(n):
            """psum region of gate bank n (0..3 = f,i,g,o)."""
            if n < 2:
                return pg01[:, 512 * n:512 * (n + 1)]
            return (pgg if n == 2 else pgo)[:]

        dec_state = {}

        from bass_rust import add_dep_helper as _add_dep
        state = {}

        def _pin_dve(op):
            if "z_tt" in state:
                _add_dep(op.ins, state["z_tt"].ins, sync=False,
                         reason="decoder DVE stays behind the z chain")
            return op

        def _pin_mm(op):
            if "zz_mm" in state:
                _add_dep(op.ins, state["zz_mm"].ins, sync=False,
                         reason="decoder MM stays behind the encoder chain")
            return op

        def decoder_piece_mm(cidx, piece):
            """Decoder matmuls: emitted right after the h-part so they fill
            the PE while ACT/DVE run the gate tail. Piece 0 also carries the
            mu/sigma head matmul of the previous chunk (5 stages, 4 slots)."""
            if piece == 0:
                if cidx >= 1:
                    pm = dpsum.tile([2 * STATE, 512], dt.float32, tag="dscr")
                    _pin_mm(nc.tensor.matmul(
                        pm[:], msw[:], dec_state["d4"][:],
                        start=True, stop=True))
                    dec_state["pm"] = pm
                p1 = dpsum.tile([64, 512], dt.float32, tag="dscr")
                zblk = azx2[:, 4 * cidx + 1:4 * cidx + 5, 0, :]
                _pin_mm(nc.tensor.matmul(p1[:], dw1z[:], zblk,
                                         start=True, stop=False))
                nc.tensor.matmul(p1[:], dw1o[:], obs[:],
                                 start=False, stop=True)
                dec_state["p"] = p1
            elif piece == 1:
                p2 = dpsum.tile([64, 512], dt.float32, tag="dscr")
                _pin_mm(nc.tensor.matmul(p2[:], dw2[:], dec_state["d1"][:],
                                         start=True, stop=True))
                dec_state["p"] = p2
            elif piece == 2:
                p3 = dpsum.tile([32, 512], dt.float32, tag="dscr")
                _pin_mm(nc.tensor.matmul(p3[:], dw3[:], dec_state["d2"][:],
                                         start=True, stop=True))
                dec_state["p"] = p3
            else:
                p4 = dpsum.tile([16, 512], dt.float32, tag="dscr")
                _pin_mm(nc.tensor.matmul(p4[:], dw4[:], dec_state["d3"][:],
                                         start=True, stop=True))
                dec_state["p"] = p4

        def decoder_piece_ew(cidx, piece):
            """Decoder elementwise: emitted after z_tt so the DVE in-order
            stream never blocks the gate tail behind a decoder matmul."""
            p = dec_state["p"]
            if piece == 0:
                if cidx >= 1:
                    cs = slice(512 * (cidx - 1), 512 * cidx)
                    ms_sb = dpool.tile([2 * STATE, 512], dt.float32,
                                       tag="ms")
                    _pin_dve(nc.vector.tensor_scalar_add(
                        ms_sb[:], dec_state["pm"][:], msb[:]))
                    nc.sync.dma_start(out=ms_o[:, cs], in_=ms_sb[:])
                d1 = dpool.tile([64, 512], dt.bfloat16, tag="d1")
                _pin_dve(nc.vector.tensor_scalar(
                    d1[:], p[:], 0.0, None, OP.max))
                dec_state["d1"] = d1
            elif piece == 1:
                d2 = dpool.tile([64, 512], dt.bfloat16, tag="d2")
                _pin_dve(nc.vector.tensor_scalar(
                    d2[:], p[:], db2[:], 0.0, OP.add, OP.max))
                dec_state["d2"] = d2
            elif piece == 2:
                d3 = dpool.tile([32, 512], dt.bfloat16, tag="d3")
                _pin_dve(nc.vector.tensor_scalar(
                    d3[:], p[:], db3[:], 0.0, OP.add, OP.max))
                dec_state["d3"] = d3
            else:
                d4 = dpool.tile([16, 512], dt.bfloat16, tag="d4")
                _pin_dve(nc.vector.tensor_scalar(
                    d4[:], p[:], db4[:], 0.0, OP.add, OP.max))
                dec_state["d4"] = d4

        def decoder_ms(cidx):
            pm = dpsum.tile([2 * STATE, 512], dt.float32, tag="dscr")
            nc.tensor.matmul(pm[:], msw[:], dec_state["d4"][:],
                             start=True, stop=True)
            ms_sb = dpool.tile([2 * STATE, 512], dt.float32, tag="ms")
            nc.vector.tensor_scalar_add(ms_sb[:], pm[:], msb[:])
            nc.sync.dma_start(out=ms_o[:, 512 * cidx:512 * (cidx + 1)],
                              in_=ms_sb[:])

        whv = wh[:].rearrange("p (k w) -> p k w", k=4)
        w1v = w1[:].rearrange("p (k w) -> p k w", k=4)

        def h_mm(t, j, n, start=False, stop=False):
            stat = h2f[:, 256 * j:256 * (j + 1)].rearrange(
                "p (two b) -> p two b", two=2)
            return nc.tensor.matmul(
                gbank(n), stat,
                whv[:, 2 * j:2 * (j + 1), 512 * n:512 * (n + 1)],
                start=start, stop=stop,
                perf_mode=DR)

        def az_mm(t, n, stop=False):
            # fused (z_{t-1}, a_t) DoubleRow accumulation
            return nc.tensor.matmul(
                gbank(n), azx2[:, t, :, :],
                wzahv[:, :, 512 * n:512 * (n + 1)],
                start=False, stop=stop,
                perf_mode=DR)

        # ---- recurrence ----
        # Per-bank accumulation groups: h j0 opens (start); banks f,i close
        # on their az matmul, banks g,o close on the h j1 matmul which is
        # DEFERRED into the next body so it streams during tanh(f,i) and
        # keeps the PE from going cold (p-state) in the gate tail.
        # gates(0): h2f and z_{-1} are zero
        for n in range(4):
            h_mm(0, 0, n, start=True)
        h_mm(0, 1, 0)
        h_mm(0, 1, 1)
        for n in range(4):
            az_mm(0, n, stop=(n < 2))
        for t in range(t_steps):
            eps_t = epool.tile([Z, BL], dt.bfloat16, tag="eps")
            nc.sync.dma_start(out=eps_t[:], in_=eps_d[t, :, :])

            # tanh over (f,i) merged; banks g,o close during it (deferred
            # h j1 matmuls keep the PE busy through the ACT phase)
            tg = gpool.tile([128, 4 * H], dt.float32, tag="tanh_g")
            nc.scalar.activation(tg[:, 0:1024], pg01[:], AF.Tanh,
                                 scale=1.0 / 64.0)
            h_mm(t, 1, 2, stop=True)
            h_mm(t, 1, 3, stop=True)
            nc.scalar.activation(tg[:, 1024:1536], pgg[:],
                                 AF.Tanh, scale=1.0 / 64.0)
            nc.scalar.activation(tg[:, 1536:2048], pgo[:],
                                 AF.Tanh, scale=1.0 / 64.0)

            t_f = tg[:, 0:512]
            t_i = tg[:, 512:1024]
            t_g = tg[:, 1024:1536]
            t_o = tg[:, 1536:2048]

            tmp1 = gpool.tile([128, H], dt.float32, tag="tmp1")
            tmp2 = gpool.tile([128, H], dt.float32, tag="tmp2")
            tcn = gpool.tile([128, H], dt.float32, tag="tanh_c")
            # tmp1 = (1+tanh(f/2)) * cD ; then per 256-half:
            # tmp2 = (1+tanh(i/2)) * g ; cD = 0.5*tmp1 + tmp2 (= 2*c_new).
            # Explicit order pin keeps the scheduler from running tmp2h1
            # ahead of cdh0 on the in-order DVE.
            h0 = slice(0, 256)
            h1 = slice(256, 512)
            nc.vector.scalar_tensor_tensor(
                tmp1[:], t_f, 1.0, cd[:], OP.add, OP.mult)
            nc.vector.scalar_tensor_tensor(
                tmp2[:, h0], t_i[:, h0], 1.0, t_g[:, h0], OP.add, OP.mult)
            cdh0 = nc.vector.scalar_tensor_tensor(
                cd[:, h0], tmp1[:, h0], 0.5, tmp2[:, h0], OP.mult, OP.add)
            tmp2h1 = nc.vector.scalar_tensor_tensor(
                tmp2[:, h1], t_i[:, h1], 1.0, t_g[:, h1], OP.add, OP.mult)
            _add_dep(tmp2h1.ins, cdh0.ins, sync=False,
                     reason="half0 c-chain completes first on the DVE")
            nc.vector.scalar_tensor_tensor(
                cd[:, h1], tmp1[:, h1], 0.5, tmp2[:, h1], OP.mult, OP.add)
            # tc = tanh(0.5*cD); h2 = (1+tanh(o/2)) * tc, in halves so the
            # transposes / h-part start on the first half early
            for hh, hx in ((0, h0), (1, h1)):
                nc.scalar.activation(tcn[:, hx], cd[:, hx], AF.Tanh,
                                     scale=0.5)
                nc.vector.scalar_tensor_tensor(
                    h2b[:, hx], t_o[:, hx], 1.0, tcn[:, hx],
                    OP.add, OP.mult)

            # transpose h2 batch-major -> feature-major on the PE;
            # psum->sbuf copies split between ACT and DVE
            for k in range(4):
                ptr = dpsum.tile([128, 128], dt.bfloat16, tag="dscr")
                nc.tensor.transpose(
                    ptr[:], h2b[:, 128 * k:128 * (k + 1)], ident[:])
                dst = h2f[:, 128 * k:128 * (k + 1)]
                if k % 2 == 0:
                    nc.scalar.copy(dst, ptr[:])
                else:
                    nc.vector.tensor_copy(dst, ptr[:])

            # encoder (feature-major): e1/e2/zz share one PSUM bank; e1 via
            # fp8 DoubleRow; h(t+1) j0 matmuls interleave into the z-path's
            # PE wait slots so the PE never idles long enough to re-throttle
            pe = epsum.tile([128, 384], dt.float32, tag="enc")
            for m in range(2):
                out = pe[:, 128 * m:128 * (m + 1)]
                for j in range(2):
                    state["e1_mm"] = nc.tensor.matmul(
                        out,
                        w1v[:, 2 * j:2 * (j + 1), 128 * m:128 * (m + 1)],
                        h2f[:, 256 * j:256 * (j + 1)].rearrange(
                            "p (two b) -> p two b", two=2),
                        start=(j == 0), stop=(j == 1),
                        perf_mode=DR)
            # e1 stored as 64*e1 in fp8 (descale folded into enc_w2)
            e1 = gpool.tile([128, 256], dt.float8e4, tag="e1")
            nc.scalar.activation(e1[:, 0:128], pe[:, 0:128], AF.Relu,
                                 bias=b1[:, 0:1])
            nc.vector.tensor_scalar(
                e1[:, 128:256], pe[:, 128:256],
                b1[:, 1:2], 0.0, OP.add, OP.max)
            if t + 1 < t_steps:
                for n in (0, 1):
                    mm = h_mm(t + 1, 0, n, start=True)
                    _add_dep(mm.ins, state["e1_mm"].ins, sync=False,
                             reason="z-path e1 streams before the h-block")
            # e2' = max(psum + 4096*b2, 0) = 4096*e2 (descale in wzz)
            out = pe[:, 256:384]
            state["e2_mm"] = nc.tensor.matmul(
                out, w2[:].rearrange("p (k m) -> p k m", k=2),
                e1[:].rearrange("p (two b) -> p two b", two=2),
                start=True, stop=True,
                perf_mode=DR)
            e2 = gpool.tile([128, 128], dt.bfloat16, tag="e2")
            nc.vector.tensor_scalar(
                e2[:], out, b2[:], 0.0, OP.add, OP.max)
            if t + 1 < t_steps:
                for n in (2, 3):
                    mm = h_mm(t + 1, 0, n, start=True)
                    _add_dep(mm.ins, state["e2_mm"].ins, sync=False,
                             reason="e2 streams before h j0 g,o")
            # zz = WZ @ e2' (weights pre-descaled); hi first: the exp->zse
            # path is the longer consumer
            zz_mm = nc.tensor.matmul(pe[:, 0:Z], wzz[:, 0:Z], e2[:],
                                     start=True, stop=True)
            zz_lo = nc.tensor.matmul(pe[:, Z:2 * Z], wzz[:, Z:2 * Z], e2[:],
                                     start=True, stop=True)
            state["zz_mm"] = zz_mm
            if t + 1 < t_steps:
                for n in (0, 1):
                    mm = h_mm(t + 1, 1, n)
                    _add_dep(mm.ins, zz_lo.ins, sync=False,
                             reason="zz streams before the h j1 block")
            if t >= 4:
                decoder_piece_mm(t // 4 - 1, t % 4)
            # z_scale = exp(zz_hi + (1+bz_hi))
            zsc = gpool.tile([Z, BL], dt.float32, tag="zsc")
            nc.scalar.activation(zsc[:], pe[:, 0:Z], AF.Exp,
                                 bias=bzhi[:])
            zse = gpool.tile([Z, BL], dt.float32, tag="zse")
            nc.vector.tensor_tensor(zse[:], zsc[:], eps_t[:], OP.mult)
            # z/8 = (zz_lo/8 + bz_lo/8) + zsc * eps/8, fp8 into the stream
            z_tt = nc.vector.scalar_tensor_tensor(
                azx2[:, t + 1, 0, :],
                pe[:, Z:2 * Z], bzlo[:], zse[:], OP.add, OP.add)
            state["z_tt"] = z_tt

            # closing gate matmuls for the next step (banks f,i close here;
            # g,o close on the deferred h j1 matmuls next body)
            if t + 1 < t_steps:
                for n in range(4):
                    az_mm(t + 1, n, stop=(n < 2))

            # decoder elementwise last: stays behind this step's chain
            if t >= 4:
                decoder_piece_ew(t // 4 - 1, t % 4)

        last = t_steps // 4 - 1
        decoder_ms(last - 1)
        for piece in range(4):
            decoder_piece_mm(last, piece)
            decoder_piece_ew(last, piece)
        decoder_ms(last)

    _split_multi_waits(nc)
    return nc


def _prep_host(inputs, t_steps):
    """Host-side weight/data prep -> per-core in_maps."""
    f32 = np.float32
    x, a = inputs["x"], inputs["a"]
    W_ih, W_hh = f32(inputs["W_ih"]), f32(inputs["W_hh"])
    b_g = f32(inputs["b_ih"]) + f32(inputs["b_hh"])

    # reorder gates (i,f,g,o) -> (f,i,g,o); scale f,i,o rows (and bias) by
    # 0.5 (g keeps scale 1: it gets a plain tanh)
    perm = np.concatenate([np.arange(H, 2 * H), np.arange(0, H),
                           np.arange(2 * H, 3 * H), np.arange(3 * H, 4 * H)])
    sc = np.ones(4 * H, f32)
    sc[:2 * H] = 0.5
    sc[3 * H:] = 0.5
    W_ih_r = W_ih[perm] * sc[:, None]
    W_hh_r = W_hh[perm] * sc[:, None]
    b_r = b_g[perm] * sc

    # the whole gate pre-activation is scaled by 64 (descaled in the tanh
    # via ACT scale); z is carried at 1/8 scale in fp8, so its weight rows
    # get an extra 8x
    wzah = np.zeros((128, 2, 4 * H), f32)
    wzah[:, 0, :] = 512.0 * W_ih_r[:, ACT:].T                      # z rows
    wzah[:ACT, 1, :] = 64.0 * W_ih_r[:, :ACT].T                    # a rows
    wzah[ACT, 1, :] = 64.0 * b_r                                   # bias row
    wzah = wzah.reshape(128, 2 * 4 * H).astype(f8)
    wh = np.ascontiguousarray(
        (32.0 * W_hh_r).T.reshape(4, 128, 4 * H)).astype(f8)
    w1 = np.ascontiguousarray(
        (32.0 * f32(inputs["enc_w1"])).T.reshape(4, 128, 256)).astype(f8)
    b1 = np.ascontiguousarray(64.0 * f32(inputs["enc_b1"]).reshape(2, 128).T)
    w2 = np.ascontiguousarray(
        (64.0 * f32(inputs["enc_w2"])).T.reshape(2, 128, 128)).astype(f8)
    b2 = (4096.0 * f32(inputs["enc_b2"])).reshape(128, 1)
    # zz consumes e2' = 4096*e2 -> weights /4096; lo half additionally /8
    enc_wz = f32(inputs["enc_wz"])
    wzz = np.concatenate([enc_wz[Z:, :] / 4096.0,
                          enc_wz[:Z, :] / (8.0 * 4096.0)], axis=0)
    wzz = np.ascontiguousarray(wzz.T).astype(bf16)                 # [128, 256]
    bzlo = (f32(inputs["enc_bz"])[:Z] / 8.0).reshape(Z, 1)
    bzhi = (1.0 + f32(inputs["enc_bz"])[Z:]).reshape(128, 1)

    dec_w1 = f32(inputs["dec_w1"])
    # z is carried at 1/8 scale -> 8x on its weight rows; obs part raw
    dw1z = np.ascontiguousarray(
        8.0 * dec_w1[:, STATE:].T).astype(f8)                      # [128, 64]
    dw1o = np.ascontiguousarray(
        np.concatenate([dec_w1[:, :STATE],
                        f32(inputs["dec_b1"])[:, None]],
                       axis=1).T).astype(bf16)                     # [22, 64]
    dw2 = np.ascontiguousarray(f32(inputs["dec_w2"]).T).astype(bf16)
    db2 = f32(inputs["dec_b2"]).reshape(64, 1)
    dw3 = np.ascontiguousarray(f32(inputs["dec_w3"]).T).astype(bf16)
    db3 = f32(inputs["dec_b3"]).reshape(32, 1)
    dw4 = np.ascontiguousarray(f32(inputs["dec_w4"]).T).astype(bf16)
    db4 = f32(inputs["dec_b4"]).reshape(16, 1)
    msw = np.ascontiguousarray(np.concatenate(
        [f32(inputs["mu_w"]), f32(inputs["sig_w"])], axis=0).T).astype(bf16)
    msb = np.concatenate(
        [f32(inputs["mu_b"]), 1.0 + f32(inputs["sig_b"])]).reshape(
        2 * STATE, 1)

    # eps via jax CPU (exact reference PRNG); carried at 1/8 scale
    import jax
    with jax.default_device(jax.devices("cpu")[0]):
        eps = np.asarray(jax.random.normal(
            jax.random.key(42), (T, B, Z), dtype=jax.numpy.float32))

    shared = dict(wzah=wzah, wh=wh, w1=w1, b1=b1, w2=w2, b2=b2, wzz=wzz,
                  bzlo=bzlo, bzhi=bzhi, dw1z=dw1z, dw1o=dw1o, dw2=dw2,
                  db2=db2, dw3=dw3, db3=db3, dw4=dw4, db4=db4, msw=msw,
                  msb=msb)

    in_maps = []
    for ci in range(NCORES):
        bs = slice(ci * BL, (ci + 1) * BL)
        # z/a stream: [z_{-1}=0 | a_0 | z_0 | a_1 | ...]; a_t slot holds
        # [a_t^T; 1; 0...] on 128 partitions, z slots start zeroed
        azx = np.zeros((128, 2 * t_steps + 2, BL), f32)
        a_c = f32(a[bs, :t_steps, :]).transpose(1, 2, 0)           # [T,ACT,BL]
        azx[:ACT, 1:2 * t_steps:2, :] = a_c.transpose(1, 0, 2)
        azx[ACT, 1:2 * t_steps:2, :] = 1.0
        azx = np.ascontiguousarray(
            azx.reshape(128, (2 * t_steps + 2) * BL)).astype(f8)
        eps_c = np.ascontiguousarray(
            (eps[:t_steps, bs, :] / 8.0).transpose(0, 2, 1)).astype(bf16)
        obs_c = f32(x[bs, 0, :]).T                                 # [21, BL]
        obs_rep = np.concatenate(
            [np.tile(obs_c, (1, 512 // BL)),
             np.ones((1, 512), f32)], axis=0).astype(bf16)
        m = dict(shared)
        m.update(azx=azx, eps=eps_c, obs_rep=obs_rep)
        in_maps.append(m)
    return in_maps


def _run(inputs, t_steps=T):
    from concourse.bass_utils import run_bass_kernel_spmd

    key = ("nc", t_steps)
    if key not in _CACHE:
        _CACHE[key] = _build_nc(t_steps)
    nc = _CACHE[key]
    in_maps = _prep_host(inputs, t_steps)
    res = run_bass_kernel_spmd(nc, in_maps, list(range(NCORES)),
                               trace=False)
    return res.results


def kernel(**inputs):
    t_steps = T
    results = _run(inputs, t_steps)

    y = np.float32(inputs["y"])
    su2 = 0.0
    ss = 0.0
    sabs = 0.0
    ssd = 0.0
    n_el = NCORES * STATE * t_steps * BL
    for ci in range(NCORES):
        bs = slice(ci * BL, (ci + 1) * BL)
        ms = results[ci]["ms_out"].astype(np.float64)     # [42, T*BL]
        mu = ms[:STATE]
        s = ms[STATE:]                                    # log(sigma)
        y_c = y[bs, :t_steps, :].transpose(2, 1, 0).reshape(
            STATE, t_steps * BL).astype(np.float64)
        sd = np.exp(s)
        u = (y_c - mu) / sd
        su2 += (u * u).sum()
        ss += s.sum()
        sabs += np.abs(mu - y_c).sum()
        ssd += sd.sum()
    n_tb = NCORES * t_steps * BL
    out1 = (0.5 * su2 + ss) / n_tb + STATE * LOG_SQRT_2PI
    out2 = sabs / n_el
    out3 = ssd / n_el
    return (np.float32(out1), np.float32(out2), np.float32(out3))


if __name__ == "__main__":
    import jax
    with jax.default_device(jax.devices("cpu")[0]):
        import reference as R
        inputs = {k: np.asarray(v) for k, v in R.setup_inputs().items()}
    out = kernel(**inputs)
    print("kernel:", [float(o) for o in out])
